# revision 15
# baseline (speedup 1.0000x reference)
"""DirectedGCNConv on 8 Trainium2 NeuronCores (Bass/Tile).

Strategy: target nodes sharded across the 8 cores, edges partitioned by
target, 64x64 weights replicated.  The symmetric norm FACTORIZES:
norm_e = dinv[s]*dinv[t], so the kernel gathers from host-prescaled
x~ = dinv * x, accumulates with a pure 0/1 one-hot scatter matmul, and
applies dinv[t] (with the final 0.5 folded in) as the per-partition scale of
the output relu.  Bias enters via a rank-1 matmul with u = sqrt(deg).

Load balancing: dst nodes are assigned to the 784 (core, tile) bins by a
capacity-constrained 2D LPT on (in-deg_fwd, in-deg_bwd) so every tile sees
~E/784 edges in BOTH directions (the bass program is shared SPMD, so chunk
counts take the max over cores -- balancing kills that padding).  Sources are
split over 5 OVERLAPPING 32768-row windows (int16 gather indices); each edge
picks a covering window greedily so windows 0..3 fill to exactly cap=2 chunks
(256 edges, zero pad) and window 4 takes the remainder.  The host unpermutes
the output rows at the end.

Device-side per core, per direction:
  - x~ rows (bf16, padded to 128 cols = 256B) fetched with dma_gather in
    1024-index calls (the HW max), round-robin over the 4 SWDGE queues.
  - the 0/1 one-hot S is built 8 chunks at a time with ONE DVE tensor_tensor
    is_equal op (iota pattern vs dl broadcast along the free dim).
  - per 128-edge chunk one TensorE matmul accumulates aggT[64f, 128d] in PSUM;
    the self loop is an identity-matmul of the (permuted, host-gathered) x~
    slice; aggT -> SBUF bf16 via ACT copy, W-matmul + bias matmul, relu with
    scale=0.5*dinv on ACT; directions summed on DVE, written out.
"""

import heapq
from contextlib import ExitStack

import ml_dtypes
import numpy as np

N_NODES = 100000
D = 64
N_CORES = 8
RPC = N_NODES // N_CORES          # 12500 target rows per core
P = 128
N_TILES = (RPC + P - 1) // P      # 98
TILE_PAD = N_TILES * P            # 12544
N_BINS = N_CORES * N_TILES        # 784 (every tile is fully used; 12500*8 = 98*128*8 - pad)
WLEN = 32768
WSTART = [0, 16808, 33616, 50424, 67232]
N_WIN = 5
CALL_CH = 16                      # chunks per dma_gather call (16*128 = 2048 idx)

BF16 = ml_dtypes.bfloat16
LAST_RESULTS = None


def _balance_nodes(degs):
    """Assign nodes to N_BINS bins of <=128 nodes so that BOTH per-direction
    degree sums stay at/below the 10-chunk boundary (1280).  Pair nodes with
    opposite deg_f - deg_b residuals (each pair ~balanced across directions),
    LPT the pairs on their total, then swap-repair bins over the cap.
    Returns slot[node] in [0, N_CORES*TILE_PAD)."""
    df = degs[0].astype(np.int64)
    db = degs[1].astype(np.int64)
    order = np.argsort(df - db, kind="stable")
    half = N_NODES // 2
    pa, pb = order[:half], order[N_NODES - half :][::-1]   # opposite residuals
    ptot = df[pa] + db[pa] + df[pb] + db[pb]

    porder = np.argsort(-ptot, kind="stable")
    heap = [(0, i) for i in range(N_BINS)]
    heapq.heapify(heap)
    counts = np.zeros(N_BINS, np.int64)
    binof = np.empty(N_NODES, np.int64)
    pair_cap = 64                                          # 128 nodes per bin
    for pi in porder:
        while True:
            load, i = heapq.heappop(heap)
            if counts[i] < pair_cap:
                break
        binof[pa[pi]] = i
        binof[pb[pi]] = i
        counts[i] += 1
        if counts[i] < pair_cap:
            heapq.heappush(heap, (load + int(ptot[pi]), i))

    # swap-repair: force lf <= CAP and lb <= CAP where possible
    CAP = 1280
    lf = np.bincount(binof, weights=df, minlength=N_BINS).astype(np.int64)
    lb = np.bincount(binof, weights=db, minlength=N_BINS).astype(np.int64)
    members = [[] for _ in range(N_BINS)]
    for n in range(N_NODES):
        members[binof[n]].append(n)
    for _ in range(4):
        viol = [i for i in range(N_BINS) if lf[i] > CAP or lb[i] > CAP]
        if not viol:
            break
        slack_bins = sorted(
            (i for i in range(N_BINS) if lf[i] < CAP - 2 and lb[i] < CAP - 2),
            key=lambda i: lf[i] + lb[i],
        )
        for i in viol:
            guard = 0
            while (lf[i] > CAP or lb[i] > CAP) and guard < 40:
                guard += 1
                use_f = lf[i] - CAP >= lb[i] - CAP
                mem = members[i]
                n_out = max(mem, key=(lambda n: df[n]) if use_f else (lambda n: db[n]))
                swapped = False
                for j in slack_bins:
                    if j == i:
                        continue
                    m_in = min(members[j], key=lambda n: df[n] + db[n])
                    nlf_j = lf[j] + df[n_out] - df[m_in]
                    nlb_j = lb[j] + db[n_out] - db[m_in]
                    if nlf_j > CAP or nlb_j > CAP:
                        continue
                    if df[m_in] >= df[n_out] and db[m_in] >= db[n_out]:
                        continue
                    members[i].remove(n_out)
                    members[j].remove(m_in)
                    members[i].append(m_in)
                    members[j].append(n_out)
                    lf[i] += df[m_in] - df[n_out]
                    lb[i] += db[m_in] - db[n_out]
                    lf[j] = nlf_j
                    lb[j] = nlb_j
                    binof[n_out] = j
                    binof[m_in] = i
                    swapped = True
                    break
                if not swapped:
                    break

    # slot within bin: arbitrary order
    slot = np.empty(N_NODES, np.int64)
    offs = np.zeros(N_BINS, np.int64)
    for n in range(N_NODES):
        i = binof[n]
        core, ti = i // N_TILES, i % N_TILES
        slot[n] = core * TILE_PAD + ti * P + offs[i]
        offs[i] += 1
    return slot


def _prep_dir(tslot, s):
    """Host-side edge partitioning for one direction.

    tslot = target slot (already permuted, in [0, N_CORES*TILE_PAD));
    s = source node id.  Window-major chunk layout, CALL_CH-aligned window
    bases."""
    E = tslot.shape[0]
    core = tslot // TILE_PAD
    tl = tslot - core * TILE_PAD
    ti = tl // P
    dl = tl - ti * P

    # --- greedy window assignment with per-(ti) caps ------------------------
    # caps: windows 0..3 take exactly 2 chunks (256), window 4 the rest.
    grp = (core * N_TILES + ti)
    order0 = np.argsort(grp * np.int64(N_NODES) + s, kind="stable")
    grp_s = grp[order0]
    s_s = s[order0]
    gcounts = np.bincount(grp, minlength=N_CORES * N_TILES)
    gstart = np.zeros(N_CORES * N_TILES + 1, np.int64)
    np.cumsum(gcounts, out=gstart[1:])

    # cumulative mandatory counts: edges with src < WSTART[w+1] must be
    # assigned to windows <= w.  Template cumulative caps (shared across
    # cores) = max over cores, rounded up to whole chunks, floor 2 chunks per
    # window.
    cum_mand = np.zeros((N_CORES * N_TILES, N_WIN), np.int64)
    for g in range(N_CORES * N_TILES):
        a, e = gstart[g], gstart[g + 1]
        src = s_s[a:e]
        for w in range(N_WIN - 1):
            cum_mand[g, w] = np.searchsorted(src, WSTART[w + 1])
        cum_mand[g, N_WIN - 1] = e - a
    cm = cum_mand.reshape(N_CORES, N_TILES, N_WIN).max(axis=0)   # [98, 5]
    cumcap = -(-cm // P) * P
    for w in range(N_WIN):
        cumcap[:, w] = np.maximum(cumcap[:, w], 2 * P * (w + 1))
    for w in range(1, N_WIN):
        cumcap[:, w] = np.maximum(cumcap[:, w], cumcap[:, w - 1] + P)
    caps_ti = np.empty((N_TILES, N_WIN), np.int64)
    caps_ti[:, 0] = cumcap[:, 0]
    caps_ti[:, 1:] = cumcap[:, 1:] - cumcap[:, :-1]
    caps_ti[:, N_WIN - 1] = 1 << 30           # last window absorbs any spill

    bk_s = np.empty(E, np.int8)
    n_gw = np.zeros((N_CORES * N_TILES, N_WIN), np.int64)
    for g in range(N_CORES * N_TILES):
        a, e = gstart[g], gstart[g + 1]
        src = s_s[a:e]                       # sorted ascending within group
        caps = caps_ti[g % N_TILES]
        pos = 0
        n = e - a
        for w in range(N_WIN):
            hi = np.searchsorted(src, WSTART[w] + WLEN)
            take = min(int(caps[w]), hi - pos)
            if w + 1 < N_WIN:
                mand = np.searchsorted(src, WSTART[w + 1]) - pos
                assert mand <= caps[w], (g, w, mand, caps[w])
            else:
                take = n - pos
            bk_s[a + pos : a + pos + take] = w
            n_gw[g, w] = take
            pos += take
        assert pos == n

    # chunk template per (w, ti): measured per-core max, shared across cores
    nch_tb = (
        -(-n_gw.reshape(N_CORES, N_TILES, N_WIN).max(axis=0) // P)
    ).T.copy()                                # [5, 98]

    tb_gbase = np.zeros((N_WIN, N_TILES), np.int64)
    chunk_ti = []
    wbase = np.zeros(N_WIN, np.int64)
    wn = np.zeros(N_WIN, np.int64)
    gc = 0
    for w in range(N_WIN):
        gc = ((gc + CALL_CH - 1) // CALL_CH) * CALL_CH
        wbase[w] = gc
        for ti_ in range(N_TILES):
            tb_gbase[w, ti_] = gc
            gc += nch_tb[w, ti_]
            chunk_ti.extend([ti_] * int(nch_tb[w, ti_]))
        wn[w] = gc - wbase[w]
    ctot = ((gc + CALL_CH - 1) // CALL_CH) * CALL_CH

    # --- per-edge slot assignment ------------------------------------------
    core_s = core[order0]
    ti_s = ti[order0]
    dl_s = dl[order0]
    key = (core_s * N_WIN + bk_s) * N_TILES + ti_s
    order1 = np.argsort(key, kind="stable")
    key_s = key[order1]
    counts = np.bincount(key, minlength=N_CORES * N_WIN * N_TILES)
    starts = np.zeros(N_CORES * N_WIN * N_TILES + 1, np.int64)
    np.cumsum(counts, out=starts[1:])
    rank = np.arange(E, dtype=np.int64) - starts[key_s]
    core_f = core_s[order1]
    w_f = bk_s[order1].astype(np.int64)
    gpos = tb_gbase[w_f, ti_s[order1]] * P + rank
    sl = (s_s[order1] - np.asarray(WSTART, np.int64)[w_f]).astype(np.int16)

    idx_arr = np.zeros((N_CORES, 128, ctot * 8), np.int16)
    dl_arr = np.full((N_CORES, 128, ctot), 255.0, BF16)
    idx_arr[core_f, gpos % 16, gpos // 16] = sl
    dl_arr[core_f, gpos % 128, gpos // 128] = dl_s[order1].astype(BF16)

    ti_of_chunk = np.full(ctot, -1, np.int64)
    pos = 0
    for w in range(N_WIN):
        nb = int(wn[w])
        ti_of_chunk[int(wbase[w]) : int(wbase[w]) + nb] = chunk_ti[pos : pos + nb]
        pos += nb
    calls = []
    for w in range(N_WIN):
        nb = int(wn[w])
        for k in range((nb + CALL_CH - 1) // CALL_CH):
            c0 = int(wbase[w]) + CALL_CH * k
            nn = min(CALL_CH, nb - CALL_CH * k)
            calls.append((w, c0, nn, int(ti_of_chunk[c0])))
    # Q7 SWDGE reads the wrapped index block from each 16-partition group
    # (one per gpsimd core) -> replicate rows 0:16 into rows 16:128.
    idx_arr[:, 16:, :] = np.tile(idx_arr[:, :16, :], (1, 7, 1))

    meta = dict(nch_tb=nch_tb, tb_gbase=tb_gbase, calls=calls, ctot=ctot)
    return idx_arr, dl_arr, meta


def _build(ctx, tc, aps, metas):
    import concourse.mybir as mybir

    nc = tc.nc
    f32 = mybir.dt.float32
    bf16 = mybir.dt.bfloat16
    i16 = mybir.dt.int16
    Alu = mybir.AluOpType
    Act = mybir.ActivationFunctionType

    cp = ctx.enter_context(tc.tile_pool(name="const", bufs=1))

    def load(name, dtype):
        ap = aps[name].ap()
        t = cp.tile(list(ap.shape), dtype, tag=name)
        nc.sync.dma_start(out=t[:], in_=ap[:])
        return t

    # idx tensors gate the first gathers -> load them first so the Q7 queues
    # start while the remaining constants stream in behind them.
    idx_t = [load("idx0", i16), load("idx1", i16)]
    dl_t = [load("dl0", bf16), load("dl1", bf16)]
    iota_t = load("iota8", bf16)
    ident_t = load("ident", bf16)
    wh_t = [load("wh0", bf16), load("wh1", bf16)]
    bh_t = [load("bh0", bf16), load("bh1", bf16)]
    u_t = [load("u0", bf16), load("u1", bf16)]
    dvh_t = [load("dvh0", f32), load("dvh1", f32)]

    xb_ap = [aps["xb0"].ap(), aps["xb1"].ap()]
    xs_ap = aps["xs"].ap()
    out_ap = aps["out"].ap()

    gp = ctx.enter_context(tc.tile_pool(name="g", bufs=11))
    s8p = ctx.enter_context(tc.tile_pool(name="s8", bufs=13))
    xlp = ctx.enter_context(tc.tile_pool(name="xl", bufs=4))
    aggp = ctx.enter_context(tc.tile_pool(name="agg", bufs=4))
    rp = ctx.enter_context(tc.tile_pool(name="r", bufs=4))
    op_ = ctx.enter_context(tc.tile_pool(name="o", bufs=3))
    ps_t = ctx.enter_context(tc.tile_pool(name="psT", bufs=4, space="PSUM"))
    ps_b = ctx.enter_context(tc.tile_pool(name="psB", bufs=2, space="PSUM"))

    # --- emit all gather calls in consumption order -------------------------
    all_calls = []
    for d in (0, 1):
        for (w, c0, nn, fti) in metas[d]["calls"]:
            all_calls.append((fti, d, w, c0, nn))
    all_calls.sort()

    G = [{}, {}]
    qctr = 0
    for (fti, d, w, c0, nn) in all_calls:
        g = gp.tile([128, nn * 128], bf16, tag="g", name="g")
        nc.gpsimd.dma_gather(
            out_ap=g[:].rearrange("p (c e) -> p c e", e=128),
            in_ap=xb_ap[d][WSTART[w] : WSTART[w] + WLEN, :],
            idxs_ap=idx_t[d][:, c0 * 8 : (c0 + nn) * 8],
            num_idxs=nn * 128,
            num_idxs_reg=nn * 128,
            elem_size=128,
            single_packet=False,
            queue_num=qctr % 4,
        )
        qctr += 1
        G[d][c0 // CALL_CH] = g

    # --- main tile loop -----------------------------------------------------
    S8 = [{}, {}]

    def get_s8(d, batch):
        t = S8[d].get(batch)
        if t is None:
            t = s8p.tile([128, 1024], bf16, tag="s8", name="s8")
            nc.vector.tensor_tensor(
                out=t[:].rearrange("p (c e) -> p c e", e=128),
                in0=iota_t[:].rearrange("p (c e) -> p c e", e=128),
                in1=dl_t[d][:, batch * 8 : batch * 8 + 8]
                .unsqueeze(2)
                .broadcast_to([128, 8, 128]),
                op=Alu.is_equal,
            )
            S8[d][batch] = t
        return t

    for ti in range(N_TILES):
        r_ = [None, None]
        xsl = xlp.tile([128, 2 * D], bf16, tag="xl")
        nc.sync.dma_start(out=xsl[:], in_=xs_ap[ti * P : (ti + 1) * P, :])
        for d in (0, 1):
            m = metas[d]
            total_ch = int(m["nch_tb"][:, ti].sum())
            psT = ps_t.tile([D, 128], f32, tag="psT")
            nc.tensor.matmul(
                out=psT[:], lhsT=xsl[:, d * D : (d + 1) * D], rhs=ident_t[:],
                start=True, stop=(total_ch == 0),
            )
            done = 0
            for w in range(N_WIN):
                n = int(m["nch_tb"][w, ti])
                base = int(m["tb_gbase"][w, ti])
                for cc in range(n):
                    gc = base + cc
                    s8 = get_s8(d, gc // 8)
                    g = G[d][gc // CALL_CH]
                    col = (gc % CALL_CH) * 128
                    scol = (gc % 8) * 128
                    done += 1
                    nc.tensor.matmul(
                        out=psT[:],
                        lhsT=g[:, col : col + D],
                        rhs=s8[:, scol : scol + 128],
                        start=False, stop=(done == total_ch),
                    )
            aggT = aggp.tile([D, 128], bf16, tag="agg")
            nc.scalar.activation(out=aggT[:], in_=psT[:], func=Act.Copy)
            psB = ps_b.tile([128, D], f32, tag="psB")
            nc.tensor.matmul(
                out=psB[:], lhsT=aggT[:], rhs=wh_t[d][:], start=True, stop=False
            )
            nc.tensor.matmul(
                out=psB[:],
                lhsT=u_t[d][:, ti * P : (ti + 1) * P],
                rhs=bh_t[d][:],
                start=False, stop=True,
            )
            r_[d] = rp.tile([128, D], f32, name=f"r{d}", tag=f"r{d}")
            nc.scalar.activation(
                out=r_[d][:], in_=psB[:], func=Act.Relu,
                scale=dvh_t[d][:, ti : ti + 1],
            )
        o = op_.tile([128, D], f32, tag="o")
        nc.vector.tensor_add(out=o[:], in0=r_[0][:], in1=r_[1][:])
        nc.sync.dma_start(
            out=out_ap[ti * P : (ti + 1) * P, :], in_=o[:, :]
        )


def kernel(x, edge_index, W_f, b_f, W_b, b_b):
    global LAST_RESULTS
    import concourse.tile as tile
    from concourse import bacc, mybir
    from concourse import bass_utils

    x = np.asarray(x, dtype=np.float32)
    ei = np.asarray(edge_index).astype(np.int64)
    W_f = np.asarray(W_f, dtype=np.float32)
    b_f = np.asarray(b_f, dtype=np.float32)
    W_b = np.asarray(W_b, dtype=np.float32)
    b_b = np.asarray(b_b, dtype=np.float32)
    src, dst = ei[0], ei[1]

    ideg_f = np.bincount(dst, minlength=N_NODES)
    ideg_b = np.bincount(src, minlength=N_NODES)
    deg_f = (ideg_f + 1).astype(np.float32)
    deg_b = (ideg_b + 1).astype(np.float32)
    dinv_f = (1.0 / np.sqrt(deg_f)).astype(np.float32)
    dinv_b = (1.0 / np.sqrt(deg_b)).astype(np.float32)
    dinvs = [dinv_f, dinv_b]
    degs = [deg_f, deg_b]

    # balanced node -> slot permutation (shared by both directions)
    slot = _balance_nodes(np.stack([ideg_f, ideg_b]))

    # direction 0 (forward): messages src -> dst; direction 1: dst -> src
    prep = [_prep_dir(slot[dst], src), _prep_dir(slot[src], dst)]
    metas = [prep[0][2], prep[1][2]]

    # pre-scaled gather sources x~ = dinv * x (bf16, padded to 128 cols)
    # and permuted per-slot arrays
    occupied = np.zeros(N_CORES * TILE_PAD, bool)
    occupied[slot] = True
    node_of_slot = np.zeros(N_CORES * TILE_PAD, np.int64)
    node_of_slot[slot] = np.arange(N_NODES)

    xb = []
    u_arr = []
    dvh = []
    xself = np.zeros((N_CORES, TILE_PAD, 2 * D), dtype=BF16)
    for d in (0, 1):
        xt = (x * dinvs[d][:, None]).astype(BF16)
        xbd = np.zeros((N_NODES, 128), dtype=BF16)
        xbd[:, :D] = xt
        xb.append(xbd)
        slot_dinv = np.where(occupied, dinvs[d][node_of_slot], 0.0).astype(np.float32)
        slot_u = np.where(occupied, np.sqrt(degs[d][node_of_slot]), 0.0)
        xs_flat = np.zeros((N_CORES * TILE_PAD, D), dtype=BF16)
        xs_flat[occupied] = xt[node_of_slot[occupied]]
        xself[:, :, d * D : (d + 1) * D] = xs_flat.reshape(N_CORES, TILE_PAD, D)
        u_arr.append(slot_u.reshape(N_CORES, 1, TILE_PAD).astype(BF16))
        dvh.append(
            (0.5 * slot_dinv).reshape(N_CORES, N_TILES, 128).transpose(0, 2, 1).copy()
        )

    iota8 = np.tile(np.arange(128, dtype=np.float32), 8).reshape(1, 1024)
    iota8 = np.broadcast_to(iota8, (128, 1024)).astype(BF16).copy()
    ident = np.eye(128, dtype=np.float32).astype(BF16)
    whs = [W_f.astype(BF16), W_b.astype(BF16)]
    bhs = [b_f.reshape(1, D).astype(BF16), b_b.reshape(1, D).astype(BF16)]

    nc = bacc.Bacc(
        "TRN2",
        target_bir_lowering=False,
        debug=False,
        enable_asserts=False,
        num_devices=N_CORES,
        num_swdge_queues=4,
        dynamic_dma_scratch_size=49152,
    )
    dt = mybir.dt
    aps = {}
    aps["iota8"] = nc.dram_tensor("iota8", [128, 1024], dt.bfloat16, kind="ExternalInput")
    aps["ident"] = nc.dram_tensor("ident", [128, 128], dt.bfloat16, kind="ExternalInput")
    aps["xs"] = nc.dram_tensor("xs", [TILE_PAD, 2 * D], dt.bfloat16, kind="ExternalInput")
    for d in (0, 1):
        ct = metas[d]["ctot"]
        aps[f"xb{d}"] = nc.dram_tensor(f"xb{d}", [N_NODES, 128], dt.bfloat16, kind="ExternalInput")
        aps[f"wh{d}"] = nc.dram_tensor(f"wh{d}", [D, D], dt.bfloat16, kind="ExternalInput")
        aps[f"bh{d}"] = nc.dram_tensor(f"bh{d}", [1, D], dt.bfloat16, kind="ExternalInput")
        aps[f"u{d}"] = nc.dram_tensor(f"u{d}", [1, TILE_PAD], dt.bfloat16, kind="ExternalInput")
        aps[f"dvh{d}"] = nc.dram_tensor(f"dvh{d}", [128, N_TILES], dt.float32, kind="ExternalInput")
        aps[f"idx{d}"] = nc.dram_tensor(f"idx{d}", [128, ct * 8], dt.int16, kind="ExternalInput")
        aps[f"dl{d}"] = nc.dram_tensor(f"dl{d}", [128, ct], dt.bfloat16, kind="ExternalInput")
    aps["out"] = nc.dram_tensor("out", [TILE_PAD, D], dt.float32, kind="ExternalOutput")

    with tile.TileContext(nc) as tc, ExitStack() as ctx:
        _build(ctx, tc, aps, metas)
    nc.compile()

    in_maps = []
    for c in range(N_CORES):
        m = {"iota8": iota8, "ident": ident, "xs": xself[c]}
        for d in (0, 1):
            idx_arr, dl_arr, _ = prep[d]
            m[f"xb{d}"] = xb[d]
            m[f"wh{d}"] = whs[d]
            m[f"bh{d}"] = bhs[d]
            m[f"u{d}"] = u_arr[d][c]
            m[f"dvh{d}"] = dvh[d][c]
            m[f"idx{d}"] = idx_arr[c]
            m[f"dl{d}"] = dl_arr[c]
        in_maps.append(m)

    LAST_RESULTS = bass_utils.run_bass_kernel_spmd(
        nc, in_maps, core_ids=list(range(N_CORES))
    )
    allout = np.concatenate([r["out"] for r in LAST_RESULTS.results], axis=0)
    return allout[slot].astype(np.float32)


# revision 16
# speedup vs baseline: 1.4777x; 1.4777x over previous
"""DirectedGCNConv on 8 Trainium2 NeuronCores (Bass/Tile).

Strategy: target nodes sharded across the 8 cores, edges partitioned by
target, 64x64 weights replicated.  The symmetric norm FACTORIZES:
norm_e = dinv[s]*dinv[t], so the kernel gathers from host-prescaled
x~ = dinv * x, accumulates with a pure 0/1 one-hot scatter matmul, and
applies dinv[t] (with the final 0.5 folded in) as the per-partition scale of
the output relu.  Bias enters via a rank-1 matmul with u = sqrt(deg).

Load balancing: dst nodes are assigned to the 784 (core, tile) bins by a
capacity-constrained 2D LPT on (in-deg_fwd, in-deg_bwd) so every tile sees
~E/784 edges in BOTH directions (the bass program is shared SPMD, so chunk
counts take the max over cores -- balancing kills that padding).  Sources are
split over 5 OVERLAPPING 32768-row windows (int16 gather indices); each edge
picks a covering window greedily so windows 0..3 fill to exactly cap=2 chunks
(256 edges, zero pad) and window 4 takes the remainder.  The host unpermutes
the output rows at the end.

Device-side per core, per direction:
  - x~ rows (bf16, padded to 128 cols = 256B) fetched with dma_gather in
    1024-index calls (the HW max), round-robin over the 4 SWDGE queues.
  - the 0/1 one-hot S is built 8 chunks at a time with ONE DVE tensor_tensor
    is_equal op (iota pattern vs dl broadcast along the free dim).
  - per 128-edge chunk one TensorE matmul accumulates aggT[64f, 128d] in PSUM;
    the self loop is an identity-matmul of the (permuted, host-gathered) x~
    slice; aggT -> SBUF bf16 via ACT copy, W-matmul + bias matmul, relu with
    scale=0.5*dinv on ACT; directions summed on DVE, written out.
"""

import heapq
from contextlib import ExitStack

import ml_dtypes
import numpy as np

N_NODES = 100000
D = 64
N_CORES = 8
RPC = N_NODES // N_CORES          # 12500 target rows per core
P = 128
N_TILES = (RPC + P - 1) // P      # 98
TILE_PAD = N_TILES * P            # 12544
N_BINS = N_CORES * N_TILES        # 784 (every tile is fully used; 12500*8 = 98*128*8 - pad)
WLEN = 32768
WSTART = [0, 16808, 33616, 50424, 67232]
N_WIN = 5
CALL_CH = 8                       # chunks per dma_gather call (8*128 = 1024 idx, HW max)

BF16 = ml_dtypes.bfloat16
LAST_RESULTS = None


def _balance_nodes(degs):
    """Assign nodes to N_BINS bins of <=128 nodes so that BOTH per-direction
    degree sums stay at/below the 10-chunk boundary (1280).  Pair nodes with
    opposite deg_f - deg_b residuals (each pair ~balanced across directions),
    LPT the pairs on their total, then swap-repair bins over the cap.
    Returns slot[node] in [0, N_CORES*TILE_PAD)."""
    df = degs[0].astype(np.int64)
    db = degs[1].astype(np.int64)
    order = np.argsort(df - db, kind="stable")
    half = N_NODES // 2
    pa, pb = order[:half], order[N_NODES - half :][::-1]   # opposite residuals
    ptot = df[pa] + db[pa] + df[pb] + db[pb]

    porder = np.argsort(-ptot, kind="stable")
    heap = [(0, i) for i in range(N_BINS)]
    heapq.heapify(heap)
    counts = np.zeros(N_BINS, np.int64)
    binof = np.empty(N_NODES, np.int64)
    pair_cap = 64                                          # 128 nodes per bin
    for pi in porder:
        while True:
            load, i = heapq.heappop(heap)
            if counts[i] < pair_cap:
                break
        binof[pa[pi]] = i
        binof[pb[pi]] = i
        counts[i] += 1
        if counts[i] < pair_cap:
            heapq.heappush(heap, (load + int(ptot[pi]), i))

    # swap-repair: force lf <= CAP and lb <= CAP where possible
    CAP = 1280
    lf = np.bincount(binof, weights=df, minlength=N_BINS).astype(np.int64)
    lb = np.bincount(binof, weights=db, minlength=N_BINS).astype(np.int64)
    members = [[] for _ in range(N_BINS)]
    for n in range(N_NODES):
        members[binof[n]].append(n)
    for _ in range(4):
        viol = [i for i in range(N_BINS) if lf[i] > CAP or lb[i] > CAP]
        if not viol:
            break
        slack_bins = sorted(
            (i for i in range(N_BINS) if lf[i] < CAP - 2 and lb[i] < CAP - 2),
            key=lambda i: lf[i] + lb[i],
        )
        for i in viol:
            guard = 0
            while (lf[i] > CAP or lb[i] > CAP) and guard < 40:
                guard += 1
                use_f = lf[i] - CAP >= lb[i] - CAP
                mem = members[i]
                n_out = max(mem, key=(lambda n: df[n]) if use_f else (lambda n: db[n]))
                swapped = False
                for j in slack_bins:
                    if j == i:
                        continue
                    m_in = min(members[j], key=lambda n: df[n] + db[n])
                    nlf_j = lf[j] + df[n_out] - df[m_in]
                    nlb_j = lb[j] + db[n_out] - db[m_in]
                    if nlf_j > CAP or nlb_j > CAP:
                        continue
                    if df[m_in] >= df[n_out] and db[m_in] >= db[n_out]:
                        continue
                    members[i].remove(n_out)
                    members[j].remove(m_in)
                    members[i].append(m_in)
                    members[j].append(n_out)
                    lf[i] += df[m_in] - df[n_out]
                    lb[i] += db[m_in] - db[n_out]
                    lf[j] = nlf_j
                    lb[j] = nlb_j
                    binof[n_out] = j
                    binof[m_in] = i
                    swapped = True
                    break
                if not swapped:
                    break

    # slot within bin: arbitrary order
    slot = np.empty(N_NODES, np.int64)
    offs = np.zeros(N_BINS, np.int64)
    for n in range(N_NODES):
        i = binof[n]
        core, ti = i // N_TILES, i % N_TILES
        slot[n] = core * TILE_PAD + ti * P + offs[i]
        offs[i] += 1
    return slot


def _prep_dir(tslot, s):
    """Host-side edge partitioning for one direction.

    tslot = target slot (already permuted, in [0, N_CORES*TILE_PAD));
    s = source node id.  Window-major chunk layout, CALL_CH-aligned window
    bases."""
    E = tslot.shape[0]
    core = tslot // TILE_PAD
    tl = tslot - core * TILE_PAD
    ti = tl // P
    dl = tl - ti * P

    # --- greedy window assignment with per-(ti) caps ------------------------
    # caps: windows 0..3 take exactly 2 chunks (256), window 4 the rest.
    grp = (core * N_TILES + ti)
    order0 = np.argsort(grp * np.int64(N_NODES) + s, kind="stable")
    grp_s = grp[order0]
    s_s = s[order0]
    gcounts = np.bincount(grp, minlength=N_CORES * N_TILES)
    gstart = np.zeros(N_CORES * N_TILES + 1, np.int64)
    np.cumsum(gcounts, out=gstart[1:])

    # cumulative mandatory counts: edges with src < WSTART[w+1] must be
    # assigned to windows <= w.  Template cumulative caps (shared across
    # cores) = max over cores, rounded up to whole chunks, floor 2 chunks per
    # window.
    cum_mand = np.zeros((N_CORES * N_TILES, N_WIN), np.int64)
    for g in range(N_CORES * N_TILES):
        a, e = gstart[g], gstart[g + 1]
        src = s_s[a:e]
        for w in range(N_WIN - 1):
            cum_mand[g, w] = np.searchsorted(src, WSTART[w + 1])
        cum_mand[g, N_WIN - 1] = e - a
    cm = cum_mand.reshape(N_CORES, N_TILES, N_WIN).max(axis=0)   # [98, 5]
    cumcap = -(-cm // P) * P
    for w in range(N_WIN):
        cumcap[:, w] = np.maximum(cumcap[:, w], 2 * P * (w + 1))
    for w in range(1, N_WIN):
        cumcap[:, w] = np.maximum(cumcap[:, w], cumcap[:, w - 1] + P)
    caps_ti = np.empty((N_TILES, N_WIN), np.int64)
    caps_ti[:, 0] = cumcap[:, 0]
    caps_ti[:, 1:] = cumcap[:, 1:] - cumcap[:, :-1]
    caps_ti[:, N_WIN - 1] = 1 << 30           # last window absorbs any spill

    bk_s = np.empty(E, np.int8)
    n_gw = np.zeros((N_CORES * N_TILES, N_WIN), np.int64)
    for g in range(N_CORES * N_TILES):
        a, e = gstart[g], gstart[g + 1]
        src = s_s[a:e]                       # sorted ascending within group
        caps = caps_ti[g % N_TILES]
        pos = 0
        n = e - a
        for w in range(N_WIN):
            hi = np.searchsorted(src, WSTART[w] + WLEN)
            take = min(int(caps[w]), hi - pos)
            if w + 1 < N_WIN:
                mand = np.searchsorted(src, WSTART[w + 1]) - pos
                assert mand <= caps[w], (g, w, mand, caps[w])
            else:
                take = n - pos
            bk_s[a + pos : a + pos + take] = w
            n_gw[g, w] = take
            pos += take
        assert pos == n

    # chunk template per (w, ti): measured per-core max, shared across cores
    nch_tb = (
        -(-n_gw.reshape(N_CORES, N_TILES, N_WIN).max(axis=0) // P)
    ).T.copy()                                # [5, 98]

    tb_gbase = np.zeros((N_WIN, N_TILES), np.int64)
    chunk_ti = []
    wbase = np.zeros(N_WIN, np.int64)
    wn = np.zeros(N_WIN, np.int64)
    gc = 0
    for w in range(N_WIN):
        gc = ((gc + CALL_CH - 1) // CALL_CH) * CALL_CH
        wbase[w] = gc
        for ti_ in range(N_TILES):
            tb_gbase[w, ti_] = gc
            gc += nch_tb[w, ti_]
            chunk_ti.extend([ti_] * int(nch_tb[w, ti_]))
        wn[w] = gc - wbase[w]
    ctot = ((gc + CALL_CH - 1) // CALL_CH) * CALL_CH

    # --- per-edge slot assignment ------------------------------------------
    core_s = core[order0]
    ti_s = ti[order0]
    dl_s = dl[order0]
    key = (core_s * N_WIN + bk_s) * N_TILES + ti_s
    order1 = np.argsort(key, kind="stable")
    key_s = key[order1]
    counts = np.bincount(key, minlength=N_CORES * N_WIN * N_TILES)
    starts = np.zeros(N_CORES * N_WIN * N_TILES + 1, np.int64)
    np.cumsum(counts, out=starts[1:])
    rank = np.arange(E, dtype=np.int64) - starts[key_s]
    core_f = core_s[order1]
    w_f = bk_s[order1].astype(np.int64)
    gpos = tb_gbase[w_f, ti_s[order1]] * P + rank
    sl = (s_s[order1] - np.asarray(WSTART, np.int64)[w_f]).astype(np.int16)

    idx_arr = np.zeros((N_CORES, 128, ctot * 8), np.int16)
    dl_arr = np.full((N_CORES, 128, ctot), 255.0, BF16)
    idx_arr[core_f, gpos % 16, gpos // 16] = sl
    dl_arr[core_f, gpos % 128, gpos // 128] = dl_s[order1].astype(BF16)

    ti_of_chunk = np.full(ctot, -1, np.int64)
    pos = 0
    for w in range(N_WIN):
        nb = int(wn[w])
        ti_of_chunk[int(wbase[w]) : int(wbase[w]) + nb] = chunk_ti[pos : pos + nb]
        pos += nb
    calls = []
    for w in range(N_WIN):
        nb = int(wn[w])
        for k in range((nb + CALL_CH - 1) // CALL_CH):
            c0 = int(wbase[w]) + CALL_CH * k
            nn = min(CALL_CH, nb - CALL_CH * k)
            calls.append((w, c0, nn, int(ti_of_chunk[c0])))
    # Q7 SWDGE reads the wrapped index block from each 16-partition group
    # (one per gpsimd core) -> replicate rows 0:16 into rows 16:128.
    idx_arr[:, 16:, :] = np.tile(idx_arr[:, :16, :], (1, 7, 1))

    meta = dict(nch_tb=nch_tb, tb_gbase=tb_gbase, calls=calls, ctot=ctot)
    return idx_arr, dl_arr, meta


def _build(ctx, tc, aps, metas):
    import concourse.mybir as mybir

    nc = tc.nc
    f32 = mybir.dt.float32
    bf16 = mybir.dt.bfloat16
    i16 = mybir.dt.int16
    Alu = mybir.AluOpType
    Act = mybir.ActivationFunctionType

    cp = ctx.enter_context(tc.tile_pool(name="const", bufs=1))

    def load(name, dtype):
        ap = aps[name].ap()
        t = cp.tile(list(ap.shape), dtype, tag=name)
        nc.sync.dma_start(out=t[:], in_=ap[:])
        return t

    # idx tensors gate the first gathers -> load them first so the Q7 queues
    # start while the remaining constants stream in behind them.
    idx_t = [load("idx0", i16), load("idx1", i16)]
    dl_t = [load("dl0", bf16), load("dl1", bf16)]
    iota_t = load("iota8", bf16)
    ident_t = load("ident", bf16)
    wh_t = [load("wh0", bf16), load("wh1", bf16)]
    bh_t = [load("bh0", bf16), load("bh1", bf16)]
    u_t = [load("u0", bf16), load("u1", bf16)]
    dvh_t = [load("dvh0", f32), load("dvh1", f32)]

    xb_ap = [aps["xb0"].ap(), aps["xb1"].ap()]
    xs_ap = aps["xs"].ap()
    out_ap = aps["out"].ap()

    gp = ctx.enter_context(tc.tile_pool(name="g", bufs=18))
    s8p = ctx.enter_context(tc.tile_pool(name="s8", bufs=15))
    xlp = ctx.enter_context(tc.tile_pool(name="xl", bufs=4))
    aggp = ctx.enter_context(tc.tile_pool(name="agg", bufs=4))
    rp = ctx.enter_context(tc.tile_pool(name="r", bufs=4))
    op_ = ctx.enter_context(tc.tile_pool(name="o", bufs=3))
    ps_t = ctx.enter_context(tc.tile_pool(name="psT", bufs=4, space="PSUM"))
    ps_b = ctx.enter_context(tc.tile_pool(name="psB", bufs=2, space="PSUM"))

    # --- emit all gather calls in consumption order -------------------------
    all_calls = []
    for d in (0, 1):
        for (w, c0, nn, fti) in metas[d]["calls"]:
            all_calls.append((fti, d, w, c0, nn))
    all_calls.sort()

    G = [{}, {}]
    qctr = 0
    for (fti, d, w, c0, nn) in all_calls:
        g = gp.tile([128, nn * 128], bf16, tag="g", name="g")
        nc.gpsimd.dma_gather(
            out_ap=g[:].rearrange("p (c e) -> p c e", e=128),
            in_ap=xb_ap[d][WSTART[w] : WSTART[w] + WLEN, :],
            idxs_ap=idx_t[d][:, c0 * 8 : (c0 + nn) * 8],
            num_idxs=nn * 128,
            num_idxs_reg=nn * 128,
            elem_size=128,
            queue_num=qctr % 4,
        )
        qctr += 1
        G[d][c0 // CALL_CH] = g

    # --- main tile loop -----------------------------------------------------
    S8 = [{}, {}]

    def get_s8(d, batch):
        t = S8[d].get(batch)
        if t is None:
            t = s8p.tile([128, 1024], bf16, tag="s8", name="s8")
            nc.vector.tensor_tensor(
                out=t[:].rearrange("p (c e) -> p c e", e=128),
                in0=iota_t[:].rearrange("p (c e) -> p c e", e=128),
                in1=dl_t[d][:, batch * 8 : batch * 8 + 8]
                .unsqueeze(2)
                .broadcast_to([128, 8, 128]),
                op=Alu.is_equal,
            )
            S8[d][batch] = t
        return t

    for ti in range(N_TILES):
        r_ = [None, None]
        xsl = xlp.tile([128, 2 * D], bf16, tag="xl")
        nc.sync.dma_start(out=xsl[:], in_=xs_ap[ti * P : (ti + 1) * P, :])
        for d in (0, 1):
            m = metas[d]
            total_ch = int(m["nch_tb"][:, ti].sum())
            psT = ps_t.tile([D, 128], f32, tag="psT")
            nc.tensor.matmul(
                out=psT[:], lhsT=xsl[:, d * D : (d + 1) * D], rhs=ident_t[:],
                start=True, stop=(total_ch == 0),
            )
            done = 0
            for w in range(N_WIN):
                n = int(m["nch_tb"][w, ti])
                base = int(m["tb_gbase"][w, ti])
                for cc in range(n):
                    gc = base + cc
                    s8 = get_s8(d, gc // 8)
                    g = G[d][gc // CALL_CH]
                    col = (gc % CALL_CH) * 128
                    scol = (gc % 8) * 128
                    done += 1
                    nc.tensor.matmul(
                        out=psT[:],
                        lhsT=g[:, col : col + D],
                        rhs=s8[:, scol : scol + 128],
                        start=False, stop=(done == total_ch),
                    )
            aggT = aggp.tile([D, 128], bf16, tag="agg")
            nc.scalar.activation(out=aggT[:], in_=psT[:], func=Act.Copy)
            psB = ps_b.tile([128, D], f32, tag="psB")
            nc.tensor.matmul(
                out=psB[:], lhsT=aggT[:], rhs=wh_t[d][:], start=True, stop=False
            )
            nc.tensor.matmul(
                out=psB[:],
                lhsT=u_t[d][:, ti * P : (ti + 1) * P],
                rhs=bh_t[d][:],
                start=False, stop=True,
            )
            r_[d] = rp.tile([128, D], f32, name=f"r{d}", tag=f"r{d}")
            nc.scalar.activation(
                out=r_[d][:], in_=psB[:], func=Act.Relu,
                scale=dvh_t[d][:, ti : ti + 1],
            )
        o = op_.tile([128, D], f32, tag="o")
        nc.vector.tensor_add(out=o[:], in0=r_[0][:], in1=r_[1][:])
        nc.sync.dma_start(
            out=out_ap[ti * P : (ti + 1) * P, :], in_=o[:, :]
        )


def kernel(x, edge_index, W_f, b_f, W_b, b_b):
    global LAST_RESULTS
    import concourse.tile as tile
    from concourse import bacc, mybir
    from concourse import bass_utils

    x = np.asarray(x, dtype=np.float32)
    ei = np.asarray(edge_index).astype(np.int64)
    W_f = np.asarray(W_f, dtype=np.float32)
    b_f = np.asarray(b_f, dtype=np.float32)
    W_b = np.asarray(W_b, dtype=np.float32)
    b_b = np.asarray(b_b, dtype=np.float32)
    src, dst = ei[0], ei[1]

    ideg_f = np.bincount(dst, minlength=N_NODES)
    ideg_b = np.bincount(src, minlength=N_NODES)
    deg_f = (ideg_f + 1).astype(np.float32)
    deg_b = (ideg_b + 1).astype(np.float32)
    dinv_f = (1.0 / np.sqrt(deg_f)).astype(np.float32)
    dinv_b = (1.0 / np.sqrt(deg_b)).astype(np.float32)
    dinvs = [dinv_f, dinv_b]
    degs = [deg_f, deg_b]

    # balanced node -> slot permutation (shared by both directions)
    slot = _balance_nodes(np.stack([ideg_f, ideg_b]))

    # direction 0 (forward): messages src -> dst; direction 1: dst -> src
    prep = [_prep_dir(slot[dst], src), _prep_dir(slot[src], dst)]
    metas = [prep[0][2], prep[1][2]]

    # pre-scaled gather sources x~ = dinv * x (bf16, padded to 128 cols)
    # and permuted per-slot arrays
    occupied = np.zeros(N_CORES * TILE_PAD, bool)
    occupied[slot] = True
    node_of_slot = np.zeros(N_CORES * TILE_PAD, np.int64)
    node_of_slot[slot] = np.arange(N_NODES)

    xb = []
    u_arr = []
    dvh = []
    xself = np.zeros((N_CORES, TILE_PAD, 2 * D), dtype=BF16)
    for d in (0, 1):
        xt = (x * dinvs[d][:, None]).astype(BF16)
        xbd = np.zeros((N_NODES, 128), dtype=BF16)
        xbd[:, :D] = xt
        xb.append(xbd)
        slot_dinv = np.where(occupied, dinvs[d][node_of_slot], 0.0).astype(np.float32)
        slot_u = np.where(occupied, np.sqrt(degs[d][node_of_slot]), 0.0)
        xs_flat = np.zeros((N_CORES * TILE_PAD, D), dtype=BF16)
        xs_flat[occupied] = xt[node_of_slot[occupied]]
        xself[:, :, d * D : (d + 1) * D] = xs_flat.reshape(N_CORES, TILE_PAD, D)
        u_arr.append(slot_u.reshape(N_CORES, 1, TILE_PAD).astype(BF16))
        dvh.append(
            (0.5 * slot_dinv).reshape(N_CORES, N_TILES, 128).transpose(0, 2, 1).copy()
        )

    iota8 = np.tile(np.arange(128, dtype=np.float32), 8).reshape(1, 1024)
    iota8 = np.broadcast_to(iota8, (128, 1024)).astype(BF16).copy()
    ident = np.eye(128, dtype=np.float32).astype(BF16)
    whs = [W_f.astype(BF16), W_b.astype(BF16)]
    bhs = [b_f.reshape(1, D).astype(BF16), b_b.reshape(1, D).astype(BF16)]

    nc = bacc.Bacc(
        "TRN2",
        target_bir_lowering=False,
        debug=False,
        enable_asserts=False,
        num_devices=N_CORES,
        num_swdge_queues=4,
        dynamic_dma_scratch_size=49152,
    )
    dt = mybir.dt
    aps = {}
    aps["iota8"] = nc.dram_tensor("iota8", [128, 1024], dt.bfloat16, kind="ExternalInput")
    aps["ident"] = nc.dram_tensor("ident", [128, 128], dt.bfloat16, kind="ExternalInput")
    aps["xs"] = nc.dram_tensor("xs", [TILE_PAD, 2 * D], dt.bfloat16, kind="ExternalInput")
    for d in (0, 1):
        ct = metas[d]["ctot"]
        aps[f"xb{d}"] = nc.dram_tensor(f"xb{d}", [N_NODES, 128], dt.bfloat16, kind="ExternalInput")
        aps[f"wh{d}"] = nc.dram_tensor(f"wh{d}", [D, D], dt.bfloat16, kind="ExternalInput")
        aps[f"bh{d}"] = nc.dram_tensor(f"bh{d}", [1, D], dt.bfloat16, kind="ExternalInput")
        aps[f"u{d}"] = nc.dram_tensor(f"u{d}", [1, TILE_PAD], dt.bfloat16, kind="ExternalInput")
        aps[f"dvh{d}"] = nc.dram_tensor(f"dvh{d}", [128, N_TILES], dt.float32, kind="ExternalInput")
        aps[f"idx{d}"] = nc.dram_tensor(f"idx{d}", [128, ct * 8], dt.int16, kind="ExternalInput")
        aps[f"dl{d}"] = nc.dram_tensor(f"dl{d}", [128, ct], dt.bfloat16, kind="ExternalInput")
    aps["out"] = nc.dram_tensor("out", [TILE_PAD, D], dt.float32, kind="ExternalOutput")

    with tile.TileContext(nc) as tc, ExitStack() as ctx:
        _build(ctx, tc, aps, metas)
    nc.compile()

    in_maps = []
    for c in range(N_CORES):
        m = {"iota8": iota8, "ident": ident, "xs": xself[c]}
        for d in (0, 1):
            idx_arr, dl_arr, _ = prep[d]
            m[f"xb{d}"] = xb[d]
            m[f"wh{d}"] = whs[d]
            m[f"bh{d}"] = bhs[d]
            m[f"u{d}"] = u_arr[d][c]
            m[f"dvh{d}"] = dvh[d][c]
            m[f"idx{d}"] = idx_arr[c]
            m[f"dl{d}"] = dl_arr[c]
        in_maps.append(m)

    LAST_RESULTS = bass_utils.run_bass_kernel_spmd(
        nc, in_maps, core_ids=list(range(N_CORES))
    )
    allout = np.concatenate([r["out"] for r in LAST_RESULTS.results], axis=0)
    return allout[slot].astype(np.float32)


# revision 17
# speedup vs baseline: 1.4816x; 1.0027x over previous
"""DirectedGCNConv on 8 Trainium2 NeuronCores (Bass/Tile).

Strategy: target nodes sharded across the 8 cores, edges partitioned by
target, 64x64 weights replicated.  The symmetric norm FACTORIZES:
norm_e = dinv[s]*dinv[t], so the kernel gathers from host-prescaled
x~ = dinv * x, accumulates with a pure 0/1 one-hot scatter matmul, and
applies dinv[t] (with the final 0.5 folded in) as the per-partition scale of
the output relu.  Bias enters via a rank-1 matmul with u = sqrt(deg).

Load balancing: dst nodes are assigned to the 784 (core, tile) bins by a
capacity-constrained 2D LPT on (in-deg_fwd, in-deg_bwd) so every tile sees
~E/784 edges in BOTH directions (the bass program is shared SPMD, so chunk
counts take the max over cores -- balancing kills that padding).  Sources are
split over 5 OVERLAPPING 32768-row windows (int16 gather indices); each edge
picks a covering window greedily so windows 0..3 fill to exactly cap=2 chunks
(256 edges, zero pad) and window 4 takes the remainder.  The host unpermutes
the output rows at the end.

Device-side per core, per direction:
  - x~ rows (bf16, padded to 128 cols = 256B) fetched with dma_gather in
    1024-index calls (the HW max), round-robin over the 4 SWDGE queues.
  - the 0/1 one-hot S is built 8 chunks at a time with ONE DVE tensor_tensor
    is_equal op (iota pattern vs dl broadcast along the free dim).
  - per 128-edge chunk one TensorE matmul accumulates aggT[64f, 128d] in PSUM;
    the self loop is an identity-matmul of the (permuted, host-gathered) x~
    slice; aggT -> SBUF bf16 via ACT copy, W-matmul + bias matmul, relu with
    scale=0.5*dinv on ACT; directions summed on DVE, written out.
"""

import heapq
from contextlib import ExitStack

import ml_dtypes
import numpy as np

N_NODES = 100000
D = 64
N_CORES = 8
RPC = N_NODES // N_CORES          # 12500 target rows per core
P = 128
N_TILES = (RPC + P - 1) // P      # 98
TILE_PAD = N_TILES * P            # 12544
N_BINS = N_CORES * N_TILES        # 784 (every tile is fully used; 12500*8 = 98*128*8 - pad)
WLEN = 32768
WSTART = [0, 16808, 33616, 50424, 67232]
N_WIN = 5
CALL_CH = 8                       # chunks per dma_gather call (8*128 = 1024 idx, HW max)

BF16 = ml_dtypes.bfloat16
LAST_RESULTS = None


def _balance_nodes(degs):
    """Assign nodes to N_BINS bins of <=128 nodes so that BOTH per-direction
    degree sums stay at/below the 10-chunk boundary (1280).  Pair nodes with
    opposite deg_f - deg_b residuals (each pair ~balanced across directions),
    LPT the pairs on their total, then swap-repair bins over the cap.
    Returns slot[node] in [0, N_CORES*TILE_PAD)."""
    df = degs[0].astype(np.int64)
    db = degs[1].astype(np.int64)
    order = np.argsort(df - db, kind="stable")
    half = N_NODES // 2
    pa, pb = order[:half], order[N_NODES - half :][::-1]   # opposite residuals
    ptot = df[pa] + db[pa] + df[pb] + db[pb]

    porder = np.argsort(-ptot, kind="stable")
    heap = [(0, i) for i in range(N_BINS)]
    heapq.heapify(heap)
    counts = np.zeros(N_BINS, np.int64)
    binof = np.empty(N_NODES, np.int64)
    pair_cap = 64                                          # 128 nodes per bin
    for pi in porder:
        while True:
            load, i = heapq.heappop(heap)
            if counts[i] < pair_cap:
                break
        binof[pa[pi]] = i
        binof[pb[pi]] = i
        counts[i] += 1
        if counts[i] < pair_cap:
            heapq.heappush(heap, (load + int(ptot[pi]), i))

    # swap-repair: force lf <= CAP and lb <= CAP where possible
    CAP = 1280
    lf = np.bincount(binof, weights=df, minlength=N_BINS).astype(np.int64)
    lb = np.bincount(binof, weights=db, minlength=N_BINS).astype(np.int64)
    members = [[] for _ in range(N_BINS)]
    for n in range(N_NODES):
        members[binof[n]].append(n)
    for _ in range(4):
        viol = [i for i in range(N_BINS) if lf[i] > CAP or lb[i] > CAP]
        if not viol:
            break
        slack_bins = sorted(
            (i for i in range(N_BINS) if lf[i] < CAP - 2 and lb[i] < CAP - 2),
            key=lambda i: lf[i] + lb[i],
        )
        for i in viol:
            guard = 0
            while (lf[i] > CAP or lb[i] > CAP) and guard < 40:
                guard += 1
                use_f = lf[i] - CAP >= lb[i] - CAP
                mem = members[i]
                n_out = max(mem, key=(lambda n: df[n]) if use_f else (lambda n: db[n]))
                swapped = False
                for j in slack_bins:
                    if j == i:
                        continue
                    m_in = min(members[j], key=lambda n: df[n] + db[n])
                    nlf_j = lf[j] + df[n_out] - df[m_in]
                    nlb_j = lb[j] + db[n_out] - db[m_in]
                    if nlf_j > CAP or nlb_j > CAP:
                        continue
                    if df[m_in] >= df[n_out] and db[m_in] >= db[n_out]:
                        continue
                    members[i].remove(n_out)
                    members[j].remove(m_in)
                    members[i].append(m_in)
                    members[j].append(n_out)
                    lf[i] += df[m_in] - df[n_out]
                    lb[i] += db[m_in] - db[n_out]
                    lf[j] = nlf_j
                    lb[j] = nlb_j
                    binof[n_out] = j
                    binof[m_in] = i
                    swapped = True
                    break
                if not swapped:
                    break

    # slot within bin: arbitrary order
    slot = np.empty(N_NODES, np.int64)
    offs = np.zeros(N_BINS, np.int64)
    for n in range(N_NODES):
        i = binof[n]
        core, ti = i // N_TILES, i % N_TILES
        slot[n] = core * TILE_PAD + ti * P + offs[i]
        offs[i] += 1
    return slot


def _prep_dir(tslot, s):
    """Host-side edge partitioning for one direction.

    tslot = target slot (already permuted, in [0, N_CORES*TILE_PAD));
    s = source node id.  Window-major chunk layout, CALL_CH-aligned window
    bases."""
    E = tslot.shape[0]
    core = tslot // TILE_PAD
    tl = tslot - core * TILE_PAD
    ti = tl // P
    dl = tl - ti * P

    # --- greedy window assignment with per-(ti) caps ------------------------
    # caps: windows 0..3 take exactly 2 chunks (256), window 4 the rest.
    grp = (core * N_TILES + ti)
    order0 = np.argsort(grp * np.int64(N_NODES) + s, kind="stable")
    grp_s = grp[order0]
    s_s = s[order0]
    gcounts = np.bincount(grp, minlength=N_CORES * N_TILES)
    gstart = np.zeros(N_CORES * N_TILES + 1, np.int64)
    np.cumsum(gcounts, out=gstart[1:])

    # cumulative mandatory counts: edges with src < WSTART[w+1] must be
    # assigned to windows <= w.  Template cumulative caps (shared across
    # cores) = max over cores, rounded up to whole chunks, floor 2 chunks per
    # window.
    cum_mand = np.zeros((N_CORES * N_TILES, N_WIN), np.int64)
    for g in range(N_CORES * N_TILES):
        a, e = gstart[g], gstart[g + 1]
        src = s_s[a:e]
        for w in range(N_WIN - 1):
            cum_mand[g, w] = np.searchsorted(src, WSTART[w + 1])
        cum_mand[g, N_WIN - 1] = e - a
    cm = cum_mand.reshape(N_CORES, N_TILES, N_WIN).max(axis=0)   # [98, 5]
    cumcap = -(-cm // P) * P
    for w in range(N_WIN):
        cumcap[:, w] = np.maximum(cumcap[:, w], 2 * P * (w + 1))
    for w in range(1, N_WIN):
        cumcap[:, w] = np.maximum(cumcap[:, w], cumcap[:, w - 1] + P)
    caps_ti = np.empty((N_TILES, N_WIN), np.int64)
    caps_ti[:, 0] = cumcap[:, 0]
    caps_ti[:, 1:] = cumcap[:, 1:] - cumcap[:, :-1]
    caps_ti[:, N_WIN - 1] = 1 << 30           # last window absorbs any spill

    bk_s = np.empty(E, np.int8)
    n_gw = np.zeros((N_CORES * N_TILES, N_WIN), np.int64)
    for g in range(N_CORES * N_TILES):
        a, e = gstart[g], gstart[g + 1]
        src = s_s[a:e]                       # sorted ascending within group
        caps = caps_ti[g % N_TILES]
        pos = 0
        n = e - a
        for w in range(N_WIN):
            hi = np.searchsorted(src, WSTART[w] + WLEN)
            take = min(int(caps[w]), hi - pos)
            if w + 1 < N_WIN:
                mand = np.searchsorted(src, WSTART[w + 1]) - pos
                assert mand <= caps[w], (g, w, mand, caps[w])
            else:
                take = n - pos
            bk_s[a + pos : a + pos + take] = w
            n_gw[g, w] = take
            pos += take
        assert pos == n

    # chunk template per (w, ti): measured per-core max, shared across cores
    nch_tb = (
        -(-n_gw.reshape(N_CORES, N_TILES, N_WIN).max(axis=0) // P)
    ).T.copy()                                # [5, 98]

    tb_gbase = np.zeros((N_WIN, N_TILES), np.int64)
    chunk_ti = []
    wbase = np.zeros(N_WIN, np.int64)
    wn = np.zeros(N_WIN, np.int64)
    gc = 0
    for w in range(N_WIN):
        gc = ((gc + CALL_CH - 1) // CALL_CH) * CALL_CH
        wbase[w] = gc
        for ti_ in range(N_TILES):
            tb_gbase[w, ti_] = gc
            gc += nch_tb[w, ti_]
            chunk_ti.extend([ti_] * int(nch_tb[w, ti_]))
        wn[w] = gc - wbase[w]
    ctot = ((gc + CALL_CH - 1) // CALL_CH) * CALL_CH

    # --- per-edge slot assignment ------------------------------------------
    core_s = core[order0]
    ti_s = ti[order0]
    dl_s = dl[order0]
    key = (core_s * N_WIN + bk_s) * N_TILES + ti_s
    order1 = np.argsort(key, kind="stable")
    key_s = key[order1]
    counts = np.bincount(key, minlength=N_CORES * N_WIN * N_TILES)
    starts = np.zeros(N_CORES * N_WIN * N_TILES + 1, np.int64)
    np.cumsum(counts, out=starts[1:])
    rank = np.arange(E, dtype=np.int64) - starts[key_s]
    core_f = core_s[order1]
    w_f = bk_s[order1].astype(np.int64)
    gpos = tb_gbase[w_f, ti_s[order1]] * P + rank
    sl = (s_s[order1] - np.asarray(WSTART, np.int64)[w_f]).astype(np.int16)

    idx_arr = np.zeros((N_CORES, 128, ctot * 8), np.int16)
    dl_arr = np.full((N_CORES, 128, ctot), 255.0, BF16)
    idx_arr[core_f, gpos % 16, gpos // 16] = sl
    dl_arr[core_f, gpos % 128, gpos // 128] = dl_s[order1].astype(BF16)

    ti_of_chunk = np.full(ctot, -1, np.int64)
    pos = 0
    for w in range(N_WIN):
        nb = int(wn[w])
        ti_of_chunk[int(wbase[w]) : int(wbase[w]) + nb] = chunk_ti[pos : pos + nb]
        pos += nb
    calls = []
    for w in range(N_WIN):
        nb = int(wn[w])
        for k in range((nb + CALL_CH - 1) // CALL_CH):
            c0 = int(wbase[w]) + CALL_CH * k
            nn = min(CALL_CH, nb - CALL_CH * k)
            calls.append((w, c0, nn, int(ti_of_chunk[c0])))
    # Q7 SWDGE reads the wrapped index block from each 16-partition group
    # (one per gpsimd core) -> replicate rows 0:16 into rows 16:128.
    idx_arr[:, 16:, :] = np.tile(idx_arr[:, :16, :], (1, 7, 1))

    meta = dict(nch_tb=nch_tb, tb_gbase=tb_gbase, calls=calls, ctot=ctot)
    return idx_arr, dl_arr, meta


def _build(ctx, tc, aps, metas):
    import concourse.mybir as mybir

    nc = tc.nc
    f32 = mybir.dt.float32
    bf16 = mybir.dt.bfloat16
    i16 = mybir.dt.int16
    Alu = mybir.AluOpType
    Act = mybir.ActivationFunctionType

    cp = ctx.enter_context(tc.tile_pool(name="const", bufs=1))

    def load(name, dtype):
        ap = aps[name].ap()
        t = cp.tile(list(ap.shape), dtype, tag=name)
        nc.sync.dma_start(out=t[:], in_=ap[:])
        return t

    # idx tensors gate the first gathers -> load them first so the Q7 queues
    # start while the remaining constants stream in behind them.
    idx_t = [load("idx0", i16), load("idx1", i16)]
    dl_t = [load("dl0", bf16), load("dl1", bf16)]
    iota_t = load("iota8", bf16)
    ident_t = load("ident", bf16)
    wh_t = [load("wh0", bf16), load("wh1", bf16)]
    bh_t = [load("bh0", bf16), load("bh1", bf16)]
    u_t = [load("u0", bf16), load("u1", bf16)]
    dvh_t = [load("dvh0", f32), load("dvh1", f32)]

    xb_ap = [aps["xb0"].ap(), aps["xb1"].ap()]
    xs_ap = aps["xs"].ap()
    out_ap = aps["out"].ap()

    gp = ctx.enter_context(tc.tile_pool(name="g", bufs=20))
    s8p = ctx.enter_context(tc.tile_pool(name="s8", bufs=16))
    xlp = ctx.enter_context(tc.tile_pool(name="xl", bufs=4))
    aggp = ctx.enter_context(tc.tile_pool(name="agg", bufs=4))
    rp = ctx.enter_context(tc.tile_pool(name="r", bufs=4))
    op_ = ctx.enter_context(tc.tile_pool(name="o", bufs=3))
    ps_t = ctx.enter_context(tc.tile_pool(name="psT", bufs=4, space="PSUM"))
    ps_b = ctx.enter_context(tc.tile_pool(name="psB", bufs=2, space="PSUM"))

    # --- emit all gather calls in consumption order -------------------------
    all_calls = []
    for d in (0, 1):
        for (w, c0, nn, fti) in metas[d]["calls"]:
            all_calls.append((fti, d, w, c0, nn))
    all_calls.sort()

    G = [{}, {}]
    qctr = 0
    for (fti, d, w, c0, nn) in all_calls:
        g = gp.tile([128, nn * 128], bf16, tag="g", name="g")
        nc.gpsimd.dma_gather(
            out_ap=g[:].rearrange("p (c e) -> p c e", e=128),
            in_ap=xb_ap[d][WSTART[w] : WSTART[w] + WLEN, :],
            idxs_ap=idx_t[d][:, c0 * 8 : (c0 + nn) * 8],
            num_idxs=nn * 128,
            num_idxs_reg=nn * 128,
            elem_size=128,
            queue_num=qctr % 4,
        )
        qctr += 1
        G[d][c0 // CALL_CH] = g

    # --- main tile loop -----------------------------------------------------
    S8 = [{}, {}]

    def get_s8(d, batch):
        t = S8[d].get(batch)
        if t is None:
            t = s8p.tile([128, 1024], bf16, tag="s8", name="s8")
            nc.vector.tensor_tensor(
                out=t[:].rearrange("p (c e) -> p c e", e=128),
                in0=iota_t[:].rearrange("p (c e) -> p c e", e=128),
                in1=dl_t[d][:, batch * 8 : batch * 8 + 8]
                .unsqueeze(2)
                .broadcast_to([128, 8, 128]),
                op=Alu.is_equal,
            )
            S8[d][batch] = t
        return t

    for ti in range(N_TILES):
        r_ = [None, None]
        xsl = xlp.tile([128, 2 * D], bf16, tag="xl")
        nc.sync.dma_start(out=xsl[:], in_=xs_ap[ti * P : (ti + 1) * P, :])
        for d in (0, 1):
            m = metas[d]
            total_ch = int(m["nch_tb"][:, ti].sum())
            psT = ps_t.tile([D, 128], f32, tag="psT")
            nc.tensor.matmul(
                out=psT[:], lhsT=xsl[:, d * D : (d + 1) * D], rhs=ident_t[:],
                start=True, stop=(total_ch == 0),
            )
            done = 0
            for w in range(N_WIN):
                n = int(m["nch_tb"][w, ti])
                base = int(m["tb_gbase"][w, ti])
                for cc in range(n):
                    gc = base + cc
                    s8 = get_s8(d, gc // 8)
                    g = G[d][gc // CALL_CH]
                    col = (gc % CALL_CH) * 128
                    scol = (gc % 8) * 128
                    done += 1
                    nc.tensor.matmul(
                        out=psT[:],
                        lhsT=g[:, col : col + D],
                        rhs=s8[:, scol : scol + 128],
                        start=False, stop=(done == total_ch),
                    )
            aggT = aggp.tile([D, 128], bf16, tag="agg")
            nc.scalar.activation(out=aggT[:], in_=psT[:], func=Act.Copy)
            psB = ps_b.tile([128, D], f32, tag="psB")
            nc.tensor.matmul(
                out=psB[:], lhsT=aggT[:], rhs=wh_t[d][:], start=True, stop=False
            )
            nc.tensor.matmul(
                out=psB[:],
                lhsT=u_t[d][:, ti * P : (ti + 1) * P],
                rhs=bh_t[d][:],
                start=False, stop=True,
            )
            r_[d] = rp.tile([128, D], f32, name=f"r{d}", tag=f"r{d}")
            nc.scalar.activation(
                out=r_[d][:], in_=psB[:], func=Act.Relu,
                scale=dvh_t[d][:, ti : ti + 1],
            )
        o = op_.tile([128, D], f32, tag="o")
        nc.vector.tensor_add(out=o[:], in0=r_[0][:], in1=r_[1][:])
        nc.sync.dma_start(
            out=out_ap[ti * P : (ti + 1) * P, :], in_=o[:, :]
        )


def kernel(x, edge_index, W_f, b_f, W_b, b_b):
    global LAST_RESULTS
    import concourse.tile as tile
    from concourse import bacc, mybir
    from concourse import bass_utils

    x = np.asarray(x, dtype=np.float32)
    ei = np.asarray(edge_index).astype(np.int64)
    W_f = np.asarray(W_f, dtype=np.float32)
    b_f = np.asarray(b_f, dtype=np.float32)
    W_b = np.asarray(W_b, dtype=np.float32)
    b_b = np.asarray(b_b, dtype=np.float32)
    src, dst = ei[0], ei[1]

    ideg_f = np.bincount(dst, minlength=N_NODES)
    ideg_b = np.bincount(src, minlength=N_NODES)
    deg_f = (ideg_f + 1).astype(np.float32)
    deg_b = (ideg_b + 1).astype(np.float32)
    dinv_f = (1.0 / np.sqrt(deg_f)).astype(np.float32)
    dinv_b = (1.0 / np.sqrt(deg_b)).astype(np.float32)
    dinvs = [dinv_f, dinv_b]
    degs = [deg_f, deg_b]

    # balanced node -> slot permutation (shared by both directions)
    slot = _balance_nodes(np.stack([ideg_f, ideg_b]))

    # direction 0 (forward): messages src -> dst; direction 1: dst -> src
    prep = [_prep_dir(slot[dst], src), _prep_dir(slot[src], dst)]
    metas = [prep[0][2], prep[1][2]]

    # pre-scaled gather sources x~ = dinv * x (bf16, padded to 128 cols)
    # and permuted per-slot arrays
    occupied = np.zeros(N_CORES * TILE_PAD, bool)
    occupied[slot] = True
    node_of_slot = np.zeros(N_CORES * TILE_PAD, np.int64)
    node_of_slot[slot] = np.arange(N_NODES)

    xb = []
    u_arr = []
    dvh = []
    xself = np.zeros((N_CORES, TILE_PAD, 2 * D), dtype=BF16)
    for d in (0, 1):
        xt = (x * dinvs[d][:, None]).astype(BF16)
        xbd = np.zeros((N_NODES, 128), dtype=BF16)
        xbd[:, :D] = xt
        xb.append(xbd)
        slot_dinv = np.where(occupied, dinvs[d][node_of_slot], 0.0).astype(np.float32)
        slot_u = np.where(occupied, np.sqrt(degs[d][node_of_slot]), 0.0)
        xs_flat = np.zeros((N_CORES * TILE_PAD, D), dtype=BF16)
        xs_flat[occupied] = xt[node_of_slot[occupied]]
        xself[:, :, d * D : (d + 1) * D] = xs_flat.reshape(N_CORES, TILE_PAD, D)
        u_arr.append(slot_u.reshape(N_CORES, 1, TILE_PAD).astype(BF16))
        dvh.append(
            (0.5 * slot_dinv).reshape(N_CORES, N_TILES, 128).transpose(0, 2, 1).copy()
        )

    iota8 = np.tile(np.arange(128, dtype=np.float32), 8).reshape(1, 1024)
    iota8 = np.broadcast_to(iota8, (128, 1024)).astype(BF16).copy()
    ident = np.eye(128, dtype=np.float32).astype(BF16)
    whs = [W_f.astype(BF16), W_b.astype(BF16)]
    bhs = [b_f.reshape(1, D).astype(BF16), b_b.reshape(1, D).astype(BF16)]

    nc = bacc.Bacc(
        "TRN2",
        target_bir_lowering=False,
        debug=False,
        enable_asserts=False,
        num_devices=N_CORES,
        num_swdge_queues=4,
        dynamic_dma_scratch_size=49152,
    )
    dt = mybir.dt
    aps = {}
    aps["iota8"] = nc.dram_tensor("iota8", [128, 1024], dt.bfloat16, kind="ExternalInput")
    aps["ident"] = nc.dram_tensor("ident", [128, 128], dt.bfloat16, kind="ExternalInput")
    aps["xs"] = nc.dram_tensor("xs", [TILE_PAD, 2 * D], dt.bfloat16, kind="ExternalInput")
    for d in (0, 1):
        ct = metas[d]["ctot"]
        aps[f"xb{d}"] = nc.dram_tensor(f"xb{d}", [N_NODES, 128], dt.bfloat16, kind="ExternalInput")
        aps[f"wh{d}"] = nc.dram_tensor(f"wh{d}", [D, D], dt.bfloat16, kind="ExternalInput")
        aps[f"bh{d}"] = nc.dram_tensor(f"bh{d}", [1, D], dt.bfloat16, kind="ExternalInput")
        aps[f"u{d}"] = nc.dram_tensor(f"u{d}", [1, TILE_PAD], dt.bfloat16, kind="ExternalInput")
        aps[f"dvh{d}"] = nc.dram_tensor(f"dvh{d}", [128, N_TILES], dt.float32, kind="ExternalInput")
        aps[f"idx{d}"] = nc.dram_tensor(f"idx{d}", [128, ct * 8], dt.int16, kind="ExternalInput")
        aps[f"dl{d}"] = nc.dram_tensor(f"dl{d}", [128, ct], dt.bfloat16, kind="ExternalInput")
    aps["out"] = nc.dram_tensor("out", [TILE_PAD, D], dt.float32, kind="ExternalOutput")

    with tile.TileContext(nc) as tc, ExitStack() as ctx:
        _build(ctx, tc, aps, metas)
    nc.compile()

    in_maps = []
    for c in range(N_CORES):
        m = {"iota8": iota8, "ident": ident, "xs": xself[c]}
        for d in (0, 1):
            idx_arr, dl_arr, _ = prep[d]
            m[f"xb{d}"] = xb[d]
            m[f"wh{d}"] = whs[d]
            m[f"bh{d}"] = bhs[d]
            m[f"u{d}"] = u_arr[d][c]
            m[f"dvh{d}"] = dvh[d][c]
            m[f"idx{d}"] = idx_arr[c]
            m[f"dl{d}"] = dl_arr[c]
        in_maps.append(m)

    LAST_RESULTS = bass_utils.run_bass_kernel_spmd(
        nc, in_maps, core_ids=list(range(N_CORES))
    )
    allout = np.concatenate([r["out"] for r in LAST_RESULTS.results], axis=0)
    return allout[slot].astype(np.float32)


# revision 19
# speedup vs baseline: 1.4939x; 1.0083x over previous
"""DirectedGCNConv on 8 Trainium2 NeuronCores (Bass/Tile).

Strategy: target nodes sharded across the 8 cores, edges partitioned by
target, 64x64 weights replicated.  The symmetric norm FACTORIZES:
norm_e = dinv[s]*dinv[t], so the kernel gathers from host-prescaled
x~ = dinv * x, accumulates with a pure 0/1 one-hot scatter matmul, and
applies dinv[t] (with the final 0.5 folded in) as the per-partition scale of
the output relu.  Bias enters via a rank-1 matmul with u = sqrt(deg).

Load balancing: dst nodes are assigned to the 784 (core, tile) bins by a
capacity-constrained 2D LPT on (in-deg_fwd, in-deg_bwd) so every tile sees
~E/784 edges in BOTH directions (the bass program is shared SPMD, so chunk
counts take the max over cores -- balancing kills that padding).  Sources are
split over 5 OVERLAPPING 32768-row windows (int16 gather indices); each edge
picks a covering window greedily so windows 0..3 fill to exactly cap=2 chunks
(256 edges, zero pad) and window 4 takes the remainder.  The host unpermutes
the output rows at the end.

Device-side per core, per direction:
  - x~ rows (bf16, padded to 128 cols = 256B) fetched with dma_gather in
    1024-index calls (the HW max), round-robin over the 4 SWDGE queues.
  - the 0/1 one-hot S is built 8 chunks at a time with ONE DVE tensor_tensor
    is_equal op (iota pattern vs dl broadcast along the free dim).
  - per 128-edge chunk one TensorE matmul accumulates aggT[64f, 128d] in PSUM;
    the self loop is an identity-matmul of the (permuted, host-gathered) x~
    slice; aggT -> SBUF bf16 via ACT copy, W-matmul + bias matmul, relu with
    scale=0.5*dinv on ACT; directions summed on DVE, written out.
"""

import heapq
from contextlib import ExitStack

import ml_dtypes
import numpy as np

N_NODES = 100000
D = 64
N_CORES = 8
RPC = N_NODES // N_CORES          # 12500 target rows per core
P = 128
N_TILES = (RPC + P - 1) // P      # 98
TILE_PAD = N_TILES * P            # 12544
N_BINS = N_CORES * N_TILES        # 784 (every tile is fully used; 12500*8 = 98*128*8 - pad)
WLEN = 32768
WSTART = [0, 16808, 33616, 50424, 67232]
N_WIN = 5
CALL_CH = 8                       # chunks per dma_gather call (8*128 = 1024 idx, HW max)

BF16 = ml_dtypes.bfloat16
LAST_RESULTS = None


def _balance_nodes(degs):
    """Assign nodes to N_BINS bins of <=128 nodes so that BOTH per-direction
    degree sums stay at/below the 10-chunk boundary (1280).  Pair nodes with
    opposite deg_f - deg_b residuals (each pair ~balanced across directions),
    LPT the pairs on their total, then swap-repair bins over the cap.
    Returns slot[node] in [0, N_CORES*TILE_PAD)."""
    df = degs[0].astype(np.int64)
    db = degs[1].astype(np.int64)
    order = np.argsort(df - db, kind="stable")
    half = N_NODES // 2
    pa, pb = order[:half], order[N_NODES - half :][::-1]   # opposite residuals
    ptot = df[pa] + db[pa] + df[pb] + db[pb]

    porder = np.argsort(-ptot, kind="stable")
    heap = [(0, i) for i in range(N_BINS)]
    heapq.heapify(heap)
    counts = np.zeros(N_BINS, np.int64)
    binof = np.empty(N_NODES, np.int64)
    pair_cap = 64                                          # 128 nodes per bin
    for pi in porder:
        while True:
            load, i = heapq.heappop(heap)
            if counts[i] < pair_cap:
                break
        binof[pa[pi]] = i
        binof[pb[pi]] = i
        counts[i] += 1
        if counts[i] < pair_cap:
            heapq.heappush(heap, (load + int(ptot[pi]), i))

    # swap-repair: force lf <= CAP and lb <= CAP where possible
    CAP = 1280
    lf = np.bincount(binof, weights=df, minlength=N_BINS).astype(np.int64)
    lb = np.bincount(binof, weights=db, minlength=N_BINS).astype(np.int64)
    members = [[] for _ in range(N_BINS)]
    for n in range(N_NODES):
        members[binof[n]].append(n)
    for _ in range(4):
        viol = [i for i in range(N_BINS) if lf[i] > CAP or lb[i] > CAP]
        if not viol:
            break
        slack_bins = sorted(
            (i for i in range(N_BINS) if lf[i] < CAP - 2 and lb[i] < CAP - 2),
            key=lambda i: lf[i] + lb[i],
        )
        for i in viol:
            guard = 0
            while (lf[i] > CAP or lb[i] > CAP) and guard < 40:
                guard += 1
                use_f = lf[i] - CAP >= lb[i] - CAP
                mem = members[i]
                n_out = max(mem, key=(lambda n: df[n]) if use_f else (lambda n: db[n]))
                swapped = False
                for j in slack_bins:
                    if j == i:
                        continue
                    m_in = min(members[j], key=lambda n: df[n] + db[n])
                    nlf_j = lf[j] + df[n_out] - df[m_in]
                    nlb_j = lb[j] + db[n_out] - db[m_in]
                    if nlf_j > CAP or nlb_j > CAP:
                        continue
                    if df[m_in] >= df[n_out] and db[m_in] >= db[n_out]:
                        continue
                    members[i].remove(n_out)
                    members[j].remove(m_in)
                    members[i].append(m_in)
                    members[j].append(n_out)
                    lf[i] += df[m_in] - df[n_out]
                    lb[i] += db[m_in] - db[n_out]
                    lf[j] = nlf_j
                    lb[j] = nlb_j
                    binof[n_out] = j
                    binof[m_in] = i
                    swapped = True
                    break
                if not swapped:
                    break

    # slot within bin: arbitrary order
    slot = np.empty(N_NODES, np.int64)
    offs = np.zeros(N_BINS, np.int64)
    for n in range(N_NODES):
        i = binof[n]
        core, ti = i // N_TILES, i % N_TILES
        slot[n] = core * TILE_PAD + ti * P + offs[i]
        offs[i] += 1
    return slot


def _prep_dir(tslot, s):
    """Host-side edge partitioning for one direction.

    tslot = target slot (already permuted, in [0, N_CORES*TILE_PAD));
    s = source node id.  Window-major chunk layout, CALL_CH-aligned window
    bases."""
    E = tslot.shape[0]
    core = tslot // TILE_PAD
    tl = tslot - core * TILE_PAD
    ti = tl // P
    dl = tl - ti * P

    # --- greedy window assignment with per-(ti) caps ------------------------
    # caps: windows 0..3 take exactly 2 chunks (256), window 4 the rest.
    grp = (core * N_TILES + ti)
    order0 = np.argsort(grp * np.int64(N_NODES) + s, kind="stable")
    grp_s = grp[order0]
    s_s = s[order0]
    gcounts = np.bincount(grp, minlength=N_CORES * N_TILES)
    gstart = np.zeros(N_CORES * N_TILES + 1, np.int64)
    np.cumsum(gcounts, out=gstart[1:])

    # cumulative mandatory counts: edges with src < WSTART[w+1] must be
    # assigned to windows <= w.  Template cumulative caps (shared across
    # cores) = max over cores, rounded up to whole chunks, floor 2 chunks per
    # window.
    cum_mand = np.zeros((N_CORES * N_TILES, N_WIN), np.int64)
    for g in range(N_CORES * N_TILES):
        a, e = gstart[g], gstart[g + 1]
        src = s_s[a:e]
        for w in range(N_WIN - 1):
            cum_mand[g, w] = np.searchsorted(src, WSTART[w + 1])
        cum_mand[g, N_WIN - 1] = e - a
    cm = cum_mand.reshape(N_CORES, N_TILES, N_WIN).max(axis=0)   # [98, 5]
    cumcap = -(-cm // P) * P
    for w in range(N_WIN):
        cumcap[:, w] = np.maximum(cumcap[:, w], 2 * P * (w + 1))
    for w in range(1, N_WIN):
        cumcap[:, w] = np.maximum(cumcap[:, w], cumcap[:, w - 1] + P)
    caps_ti = np.empty((N_TILES, N_WIN), np.int64)
    caps_ti[:, 0] = cumcap[:, 0]
    caps_ti[:, 1:] = cumcap[:, 1:] - cumcap[:, :-1]
    caps_ti[:, N_WIN - 1] = 1 << 30           # last window absorbs any spill

    bk_s = np.empty(E, np.int8)
    n_gw = np.zeros((N_CORES * N_TILES, N_WIN), np.int64)
    for g in range(N_CORES * N_TILES):
        a, e = gstart[g], gstart[g + 1]
        src = s_s[a:e]                       # sorted ascending within group
        caps = caps_ti[g % N_TILES]
        pos = 0
        n = e - a
        for w in range(N_WIN):
            hi = np.searchsorted(src, WSTART[w] + WLEN)
            take = min(int(caps[w]), hi - pos)
            if w + 1 < N_WIN:
                mand = np.searchsorted(src, WSTART[w + 1]) - pos
                assert mand <= caps[w], (g, w, mand, caps[w])
            else:
                take = n - pos
            bk_s[a + pos : a + pos + take] = w
            n_gw[g, w] = take
            pos += take
        assert pos == n

    # chunk template per (w, ti): measured per-core max, shared across cores
    nch_tb = (
        -(-n_gw.reshape(N_CORES, N_TILES, N_WIN).max(axis=0) // P)
    ).T.copy()                                # [5, 98]

    tb_gbase = np.zeros((N_WIN, N_TILES), np.int64)
    chunk_ti = []
    wbase = np.zeros(N_WIN, np.int64)
    wn = np.zeros(N_WIN, np.int64)
    gc = 0
    for w in range(N_WIN):
        gc = ((gc + CALL_CH - 1) // CALL_CH) * CALL_CH
        wbase[w] = gc
        for ti_ in range(N_TILES):
            tb_gbase[w, ti_] = gc
            gc += nch_tb[w, ti_]
            chunk_ti.extend([ti_] * int(nch_tb[w, ti_]))
        wn[w] = gc - wbase[w]
    ctot = ((gc + CALL_CH - 1) // CALL_CH) * CALL_CH

    # --- per-edge slot assignment ------------------------------------------
    core_s = core[order0]
    ti_s = ti[order0]
    dl_s = dl[order0]
    key = (core_s * N_WIN + bk_s) * N_TILES + ti_s
    order1 = np.argsort(key, kind="stable")
    key_s = key[order1]
    counts = np.bincount(key, minlength=N_CORES * N_WIN * N_TILES)
    starts = np.zeros(N_CORES * N_WIN * N_TILES + 1, np.int64)
    np.cumsum(counts, out=starts[1:])
    rank = np.arange(E, dtype=np.int64) - starts[key_s]
    core_f = core_s[order1]
    w_f = bk_s[order1].astype(np.int64)
    gpos = tb_gbase[w_f, ti_s[order1]] * P + rank
    sl = (s_s[order1] - np.asarray(WSTART, np.int64)[w_f]).astype(np.int16)

    idx_arr = np.zeros((N_CORES, 128, ctot * 8), np.int16)
    dl_arr = np.full((N_CORES, 128, ctot), 255.0, BF16)
    idx_arr[core_f, gpos % 16, gpos // 16] = sl
    dl_arr[core_f, gpos % 128, gpos // 128] = dl_s[order1].astype(BF16)

    ti_of_chunk = np.full(ctot, -1, np.int64)
    pos = 0
    for w in range(N_WIN):
        nb = int(wn[w])
        ti_of_chunk[int(wbase[w]) : int(wbase[w]) + nb] = chunk_ti[pos : pos + nb]
        pos += nb
    calls = []
    for w in range(N_WIN):
        nb = int(wn[w])
        for k in range((nb + CALL_CH - 1) // CALL_CH):
            c0 = int(wbase[w]) + CALL_CH * k
            nn = min(CALL_CH, nb - CALL_CH * k)
            calls.append((w, c0, nn, int(ti_of_chunk[c0])))
    # Q7 SWDGE reads the wrapped index block from each 16-partition group
    # (one per gpsimd core) -> replicate rows 0:16 into rows 16:128.
    idx_arr[:, 16:, :] = np.tile(idx_arr[:, :16, :], (1, 7, 1))

    meta = dict(nch_tb=nch_tb, tb_gbase=tb_gbase, calls=calls, ctot=ctot)
    return idx_arr, dl_arr, meta


def _build(ctx, tc, aps, metas):
    import concourse.mybir as mybir

    nc = tc.nc
    f32 = mybir.dt.float32
    bf16 = mybir.dt.bfloat16
    i16 = mybir.dt.int16
    Alu = mybir.AluOpType
    Act = mybir.ActivationFunctionType

    cp = ctx.enter_context(tc.tile_pool(name="const", bufs=1))

    def load(name, dtype):
        ap = aps[name].ap()
        t = cp.tile(list(ap.shape), dtype, tag=name)
        nc.sync.dma_start(out=t[:], in_=ap[:])
        return t

    # idx tensors gate the first gathers -> load their head segment first so
    # the Q7 queues start while the remaining constants stream in behind.
    HEAD = 1536                      # cols = first ~24 gather calls

    def load_head(name, dtype):
        ap = aps[name].ap()
        t = cp.tile(list(ap.shape), dtype, tag=name)
        c = min(HEAD, ap.shape[1])
        nc.sync.dma_start(out=t[:, :c], in_=ap[:, :c])
        return t, ap, c

    idx0, idx0_ap, c0_ = load_head("idx0", i16)
    idx1, idx1_ap, c1_ = load_head("idx1", i16)
    idx_t = [idx0, idx1]
    dl_t = [load("dl0", bf16), load("dl1", bf16)]
    iota_t = load("iota8", bf16)
    ident_t = load("ident", bf16)
    nc.sync.dma_start(out=idx0[:, c0_:], in_=idx0_ap[:, c0_:])
    nc.sync.dma_start(out=idx1[:, c1_:], in_=idx1_ap[:, c1_:])
    wh_t = [load("wh0", bf16), load("wh1", bf16)]
    bh_t = [load("bh0", bf16), load("bh1", bf16)]
    u_t = [load("u0", bf16), load("u1", bf16)]
    dvh_t = [load("dvh0", f32), load("dvh1", f32)]

    xb_ap = [aps["xb0"].ap(), aps["xb1"].ap()]
    xs_ap = aps["xs"].ap()
    out_ap = aps["out"].ap()

    gp = ctx.enter_context(tc.tile_pool(name="g", bufs=20))
    s8p = ctx.enter_context(tc.tile_pool(name="s8", bufs=16))
    xlp = ctx.enter_context(tc.tile_pool(name="xl", bufs=4))
    aggp = ctx.enter_context(tc.tile_pool(name="agg", bufs=4))
    rp = ctx.enter_context(tc.tile_pool(name="r", bufs=4))
    op_ = ctx.enter_context(tc.tile_pool(name="o", bufs=3))
    ps_t = ctx.enter_context(tc.tile_pool(name="psT", bufs=4, space="PSUM"))
    ps_b = ctx.enter_context(tc.tile_pool(name="psB", bufs=2, space="PSUM"))

    # --- emit all gather calls in consumption order -------------------------
    all_calls = []
    for d in (0, 1):
        for (w, c0, nn, fti) in metas[d]["calls"]:
            all_calls.append((fti, d, w, c0, nn))
    all_calls.sort()

    # one shared register per distinct num_idxs value: a fresh to_reg per call
    # would put 250 MOVEs on the serial Pool stream (~15us)
    nregs = {}

    def nreg(n):
        r = nregs.get(n)
        if r is None:
            r = nc.gpsimd.to_reg(n)
            nregs[n] = r
        return r

    G = [{}, {}]
    qctr = 0
    for (fti, d, w, c0, nn) in all_calls:
        g = gp.tile([128, nn * 128], bf16, tag="g", name="g")
        nc.gpsimd.dma_gather(
            out_ap=g[:].rearrange("p (c e) -> p c e", e=128),
            in_ap=xb_ap[d][WSTART[w] : WSTART[w] + WLEN, :],
            idxs_ap=idx_t[d][:, c0 * 8 : (c0 + nn) * 8],
            num_idxs=nn * 128,
            num_idxs_reg=nreg(nn * 128),
            elem_size=128,
            queue_num=qctr % 4,
        )
        qctr += 1
        G[d][c0 // CALL_CH] = g

    # --- main tile loop -----------------------------------------------------
    S8 = [{}, {}]

    def get_s8(d, batch):
        t = S8[d].get(batch)
        if t is None:
            t = s8p.tile([128, 1024], bf16, tag="s8", name="s8")
            nc.vector.tensor_tensor(
                out=t[:].rearrange("p (c e) -> p c e", e=128),
                in0=iota_t[:].rearrange("p (c e) -> p c e", e=128),
                in1=dl_t[d][:, batch * 8 : batch * 8 + 8]
                .unsqueeze(2)
                .broadcast_to([128, 8, 128]),
                op=Alu.is_equal,
            )
            S8[d][batch] = t
        return t

    for ti in range(N_TILES):
        r_ = [None, None]
        xsl = xlp.tile([128, 2 * D], bf16, tag="xl")
        nc.sync.dma_start(out=xsl[:], in_=xs_ap[ti * P : (ti + 1) * P, :])
        for d in (0, 1):
            m = metas[d]
            total_ch = int(m["nch_tb"][:, ti].sum())
            psT = ps_t.tile([D, 128], f32, tag="psT")
            nc.tensor.matmul(
                out=psT[:], lhsT=xsl[:, d * D : (d + 1) * D], rhs=ident_t[:],
                start=True, stop=(total_ch == 0),
            )
            done = 0
            for w in range(N_WIN):
                n = int(m["nch_tb"][w, ti])
                base = int(m["tb_gbase"][w, ti])
                for cc in range(n):
                    gc = base + cc
                    s8 = get_s8(d, gc // 8)
                    g = G[d][gc // CALL_CH]
                    col = (gc % CALL_CH) * 128
                    scol = (gc % 8) * 128
                    done += 1
                    nc.tensor.matmul(
                        out=psT[:],
                        lhsT=g[:, col : col + D],
                        rhs=s8[:, scol : scol + 128],
                        start=False, stop=(done == total_ch),
                    )
            aggT = aggp.tile([D, 128], bf16, tag="agg")
            nc.scalar.activation(out=aggT[:], in_=psT[:], func=Act.Copy)
            psB = ps_b.tile([128, D], f32, tag="psB")
            nc.tensor.matmul(
                out=psB[:], lhsT=aggT[:], rhs=wh_t[d][:], start=True, stop=False
            )
            nc.tensor.matmul(
                out=psB[:],
                lhsT=u_t[d][:, ti * P : (ti + 1) * P],
                rhs=bh_t[d][:],
                start=False, stop=True,
            )
            r_[d] = rp.tile([128, D], f32, name=f"r{d}", tag=f"r{d}")
            nc.scalar.activation(
                out=r_[d][:], in_=psB[:], func=Act.Relu,
                scale=dvh_t[d][:, ti : ti + 1],
            )
        o = op_.tile([128, D], f32, tag="o")
        nc.vector.tensor_add(out=o[:], in0=r_[0][:], in1=r_[1][:])
        nc.sync.dma_start(
            out=out_ap[ti * P : (ti + 1) * P, :], in_=o[:, :]
        )


def kernel(x, edge_index, W_f, b_f, W_b, b_b):
    global LAST_RESULTS
    import concourse.tile as tile
    from concourse import bacc, mybir
    from concourse import bass_utils

    x = np.asarray(x, dtype=np.float32)
    ei = np.asarray(edge_index).astype(np.int64)
    W_f = np.asarray(W_f, dtype=np.float32)
    b_f = np.asarray(b_f, dtype=np.float32)
    W_b = np.asarray(W_b, dtype=np.float32)
    b_b = np.asarray(b_b, dtype=np.float32)
    src, dst = ei[0], ei[1]

    ideg_f = np.bincount(dst, minlength=N_NODES)
    ideg_b = np.bincount(src, minlength=N_NODES)
    deg_f = (ideg_f + 1).astype(np.float32)
    deg_b = (ideg_b + 1).astype(np.float32)
    dinv_f = (1.0 / np.sqrt(deg_f)).astype(np.float32)
    dinv_b = (1.0 / np.sqrt(deg_b)).astype(np.float32)
    dinvs = [dinv_f, dinv_b]
    degs = [deg_f, deg_b]

    # balanced node -> slot permutation (shared by both directions)
    slot = _balance_nodes(np.stack([ideg_f, ideg_b]))

    # direction 0 (forward): messages src -> dst; direction 1: dst -> src
    prep = [_prep_dir(slot[dst], src), _prep_dir(slot[src], dst)]
    metas = [prep[0][2], prep[1][2]]

    # pre-scaled gather sources x~ = dinv * x (bf16, padded to 128 cols)
    # and permuted per-slot arrays
    occupied = np.zeros(N_CORES * TILE_PAD, bool)
    occupied[slot] = True
    node_of_slot = np.zeros(N_CORES * TILE_PAD, np.int64)
    node_of_slot[slot] = np.arange(N_NODES)

    xb = []
    u_arr = []
    dvh = []
    xself = np.zeros((N_CORES, TILE_PAD, 2 * D), dtype=BF16)
    for d in (0, 1):
        xt = (x * dinvs[d][:, None]).astype(BF16)
        xbd = np.zeros((N_NODES, 128), dtype=BF16)
        xbd[:, :D] = xt
        xb.append(xbd)
        slot_dinv = np.where(occupied, dinvs[d][node_of_slot], 0.0).astype(np.float32)
        slot_u = np.where(occupied, np.sqrt(degs[d][node_of_slot]), 0.0)
        xs_flat = np.zeros((N_CORES * TILE_PAD, D), dtype=BF16)
        xs_flat[occupied] = xt[node_of_slot[occupied]]
        xself[:, :, d * D : (d + 1) * D] = xs_flat.reshape(N_CORES, TILE_PAD, D)
        u_arr.append(slot_u.reshape(N_CORES, 1, TILE_PAD).astype(BF16))
        dvh.append(
            (0.5 * slot_dinv).reshape(N_CORES, N_TILES, 128).transpose(0, 2, 1).copy()
        )

    iota8 = np.tile(np.arange(128, dtype=np.float32), 8).reshape(1, 1024)
    iota8 = np.broadcast_to(iota8, (128, 1024)).astype(BF16).copy()
    ident = np.eye(128, dtype=np.float32).astype(BF16)
    whs = [W_f.astype(BF16), W_b.astype(BF16)]
    bhs = [b_f.reshape(1, D).astype(BF16), b_b.reshape(1, D).astype(BF16)]

    nc = bacc.Bacc(
        "TRN2",
        target_bir_lowering=False,
        debug=False,
        enable_asserts=False,
        num_devices=N_CORES,
        num_swdge_queues=4,
        dynamic_dma_scratch_size=49152,
    )
    dt = mybir.dt
    aps = {}
    aps["iota8"] = nc.dram_tensor("iota8", [128, 1024], dt.bfloat16, kind="ExternalInput")
    aps["ident"] = nc.dram_tensor("ident", [128, 128], dt.bfloat16, kind="ExternalInput")
    aps["xs"] = nc.dram_tensor("xs", [TILE_PAD, 2 * D], dt.bfloat16, kind="ExternalInput")
    for d in (0, 1):
        ct = metas[d]["ctot"]
        aps[f"xb{d}"] = nc.dram_tensor(f"xb{d}", [N_NODES, 128], dt.bfloat16, kind="ExternalInput")
        aps[f"wh{d}"] = nc.dram_tensor(f"wh{d}", [D, D], dt.bfloat16, kind="ExternalInput")
        aps[f"bh{d}"] = nc.dram_tensor(f"bh{d}", [1, D], dt.bfloat16, kind="ExternalInput")
        aps[f"u{d}"] = nc.dram_tensor(f"u{d}", [1, TILE_PAD], dt.bfloat16, kind="ExternalInput")
        aps[f"dvh{d}"] = nc.dram_tensor(f"dvh{d}", [128, N_TILES], dt.float32, kind="ExternalInput")
        aps[f"idx{d}"] = nc.dram_tensor(f"idx{d}", [128, ct * 8], dt.int16, kind="ExternalInput")
        aps[f"dl{d}"] = nc.dram_tensor(f"dl{d}", [128, ct], dt.bfloat16, kind="ExternalInput")
    aps["out"] = nc.dram_tensor("out", [TILE_PAD, D], dt.float32, kind="ExternalOutput")

    with tile.TileContext(nc) as tc, ExitStack() as ctx:
        _build(ctx, tc, aps, metas)
    nc.compile()

    in_maps = []
    for c in range(N_CORES):
        m = {"iota8": iota8, "ident": ident, "xs": xself[c]}
        for d in (0, 1):
            idx_arr, dl_arr, _ = prep[d]
            m[f"xb{d}"] = xb[d]
            m[f"wh{d}"] = whs[d]
            m[f"bh{d}"] = bhs[d]
            m[f"u{d}"] = u_arr[d][c]
            m[f"dvh{d}"] = dvh[d][c]
            m[f"idx{d}"] = idx_arr[c]
            m[f"dl{d}"] = dl_arr[c]
        in_maps.append(m)

    LAST_RESULTS = bass_utils.run_bass_kernel_spmd(
        nc, in_maps, core_ids=list(range(N_CORES))
    )
    allout = np.concatenate([r["out"] for r in LAST_RESULTS.results], axis=0)
    return allout[slot].astype(np.float32)


# revision 21
# speedup vs baseline: 1.5142x; 1.0136x over previous
"""DirectedGCNConv on 8 Trainium2 NeuronCores (Bass/Tile).

Strategy: target nodes sharded across the 8 cores, edges partitioned by
target, 64x64 weights replicated.  The symmetric norm FACTORIZES:
norm_e = dinv[s]*dinv[t], so the kernel gathers from host-prescaled
x~ = dinv * x, accumulates with a pure 0/1 one-hot scatter matmul, and
applies dinv[t] (with the final 0.5 folded in) as the per-partition scale of
the output relu.  Bias enters via a rank-1 matmul with u = sqrt(deg).

Load balancing: dst nodes are assigned to the 784 (core, tile) bins by a
capacity-constrained 2D LPT on (in-deg_fwd, in-deg_bwd) so every tile sees
~E/784 edges in BOTH directions (the bass program is shared SPMD, so chunk
counts take the max over cores -- balancing kills that padding).  Sources are
split over 5 OVERLAPPING 32768-row windows (int16 gather indices); each edge
picks a covering window greedily so windows 0..3 fill to exactly cap=2 chunks
(256 edges, zero pad) and window 4 takes the remainder.  The host unpermutes
the output rows at the end.

Device-side per core, per direction:
  - x~ rows (bf16, padded to 128 cols = 256B) fetched with dma_gather in
    1024-index calls (the HW max), round-robin over the 4 SWDGE queues.
  - the 0/1 one-hot S is built 8 chunks at a time with ONE DVE tensor_tensor
    is_equal op (iota pattern vs dl broadcast along the free dim).
  - per 128-edge chunk one TensorE matmul accumulates aggT[64f, 128d] in PSUM;
    the self loop is an identity-matmul of the (permuted, host-gathered) x~
    slice; aggT -> SBUF bf16 via ACT copy, W-matmul + bias matmul, relu with
    scale=0.5*dinv on ACT; directions summed on DVE, written out.
"""

import heapq
from contextlib import ExitStack

import ml_dtypes
import numpy as np

N_NODES = 100000
D = 64
N_CORES = 8
RPC = N_NODES // N_CORES          # 12500 target rows per core
P = 128
N_TILES = (RPC + P - 1) // P      # 98
TILE_PAD = N_TILES * P            # 12544
N_BINS = N_CORES * N_TILES        # 784 (every tile is fully used; 12500*8 = 98*128*8 - pad)
WLEN = 32768
WSTART = [0, 16808, 33616, 50424, 67232]
N_WIN = 5
CALL_CH = 8                       # chunks per dma_gather call (8*128 = 1024 idx, HW max)
HEAD_COLS = 1536                  # idx head tensor cols (first ~24 calls/dir)

BF16 = ml_dtypes.bfloat16
LAST_RESULTS = None


def _balance_nodes(degs):
    """Assign nodes to N_BINS bins of <=128 nodes so that BOTH per-direction
    degree sums stay at/below the 10-chunk boundary (1280).  Pair nodes with
    opposite deg_f - deg_b residuals (each pair ~balanced across directions),
    LPT the pairs on their total, then swap-repair bins over the cap.
    Returns slot[node] in [0, N_CORES*TILE_PAD)."""
    df = degs[0].astype(np.int64)
    db = degs[1].astype(np.int64)
    order = np.argsort(df - db, kind="stable")
    half = N_NODES // 2
    pa, pb = order[:half], order[N_NODES - half :][::-1]   # opposite residuals
    ptot = df[pa] + db[pa] + df[pb] + db[pb]

    porder = np.argsort(-ptot, kind="stable")
    heap = [(0, i) for i in range(N_BINS)]
    heapq.heapify(heap)
    counts = np.zeros(N_BINS, np.int64)
    binof = np.empty(N_NODES, np.int64)
    pair_cap = 64                                          # 128 nodes per bin
    for pi in porder:
        while True:
            load, i = heapq.heappop(heap)
            if counts[i] < pair_cap:
                break
        binof[pa[pi]] = i
        binof[pb[pi]] = i
        counts[i] += 1
        if counts[i] < pair_cap:
            heapq.heappush(heap, (load + int(ptot[pi]), i))

    # swap-repair: force lf <= CAP and lb <= CAP where possible
    CAP = 1280
    lf = np.bincount(binof, weights=df, minlength=N_BINS).astype(np.int64)
    lb = np.bincount(binof, weights=db, minlength=N_BINS).astype(np.int64)
    members = [[] for _ in range(N_BINS)]
    for n in range(N_NODES):
        members[binof[n]].append(n)
    for _ in range(4):
        viol = [i for i in range(N_BINS) if lf[i] > CAP or lb[i] > CAP]
        if not viol:
            break
        slack_bins = sorted(
            (i for i in range(N_BINS) if lf[i] < CAP - 2 and lb[i] < CAP - 2),
            key=lambda i: lf[i] + lb[i],
        )
        for i in viol:
            guard = 0
            while (lf[i] > CAP or lb[i] > CAP) and guard < 40:
                guard += 1
                use_f = lf[i] - CAP >= lb[i] - CAP
                mem = members[i]
                n_out = max(mem, key=(lambda n: df[n]) if use_f else (lambda n: db[n]))
                swapped = False
                for j in slack_bins:
                    if j == i:
                        continue
                    m_in = min(members[j], key=lambda n: df[n] + db[n])
                    nlf_j = lf[j] + df[n_out] - df[m_in]
                    nlb_j = lb[j] + db[n_out] - db[m_in]
                    if nlf_j > CAP or nlb_j > CAP:
                        continue
                    if df[m_in] >= df[n_out] and db[m_in] >= db[n_out]:
                        continue
                    members[i].remove(n_out)
                    members[j].remove(m_in)
                    members[i].append(m_in)
                    members[j].append(n_out)
                    lf[i] += df[m_in] - df[n_out]
                    lb[i] += db[m_in] - db[n_out]
                    lf[j] = nlf_j
                    lb[j] = nlb_j
                    binof[n_out] = j
                    binof[m_in] = i
                    swapped = True
                    break
                if not swapped:
                    break

    # slot within bin: arbitrary order
    slot = np.empty(N_NODES, np.int64)
    offs = np.zeros(N_BINS, np.int64)
    for n in range(N_NODES):
        i = binof[n]
        core, ti = i // N_TILES, i % N_TILES
        slot[n] = core * TILE_PAD + ti * P + offs[i]
        offs[i] += 1
    return slot


def _prep_dir(tslot, s):
    """Host-side edge partitioning for one direction.

    tslot = target slot (already permuted, in [0, N_CORES*TILE_PAD));
    s = source node id.  Window-major chunk layout, CALL_CH-aligned window
    bases."""
    E = tslot.shape[0]
    core = tslot // TILE_PAD
    tl = tslot - core * TILE_PAD
    ti = tl // P
    dl = tl - ti * P

    # --- greedy window assignment with per-(ti) caps ------------------------
    # caps: windows 0..3 take exactly 2 chunks (256), window 4 the rest.
    grp = (core * N_TILES + ti)
    order0 = np.argsort(grp * np.int64(N_NODES) + s, kind="stable")
    grp_s = grp[order0]
    s_s = s[order0]
    gcounts = np.bincount(grp, minlength=N_CORES * N_TILES)
    gstart = np.zeros(N_CORES * N_TILES + 1, np.int64)
    np.cumsum(gcounts, out=gstart[1:])

    # cumulative mandatory counts: edges with src < WSTART[w+1] must be
    # assigned to windows <= w.  Template cumulative caps (shared across
    # cores) = max over cores, rounded up to whole chunks, floor 2 chunks per
    # window.
    cum_mand = np.zeros((N_CORES * N_TILES, N_WIN), np.int64)
    for g in range(N_CORES * N_TILES):
        a, e = gstart[g], gstart[g + 1]
        src = s_s[a:e]
        for w in range(N_WIN - 1):
            cum_mand[g, w] = np.searchsorted(src, WSTART[w + 1])
        cum_mand[g, N_WIN - 1] = e - a
    cm = cum_mand.reshape(N_CORES, N_TILES, N_WIN).max(axis=0)   # [98, 5]
    cumcap = -(-cm // P) * P
    for w in range(N_WIN):
        cumcap[:, w] = np.maximum(cumcap[:, w], 2 * P * (w + 1))
    for w in range(1, N_WIN):
        cumcap[:, w] = np.maximum(cumcap[:, w], cumcap[:, w - 1] + P)
    caps_ti = np.empty((N_TILES, N_WIN), np.int64)
    caps_ti[:, 0] = cumcap[:, 0]
    caps_ti[:, 1:] = cumcap[:, 1:] - cumcap[:, :-1]
    caps_ti[:, N_WIN - 1] = 1 << 30           # last window absorbs any spill

    bk_s = np.empty(E, np.int8)
    n_gw = np.zeros((N_CORES * N_TILES, N_WIN), np.int64)
    for g in range(N_CORES * N_TILES):
        a, e = gstart[g], gstart[g + 1]
        src = s_s[a:e]                       # sorted ascending within group
        caps = caps_ti[g % N_TILES]
        pos = 0
        n = e - a
        for w in range(N_WIN):
            hi = np.searchsorted(src, WSTART[w] + WLEN)
            take = min(int(caps[w]), hi - pos)
            if w + 1 < N_WIN:
                mand = np.searchsorted(src, WSTART[w + 1]) - pos
                assert mand <= caps[w], (g, w, mand, caps[w])
            else:
                take = n - pos
            bk_s[a + pos : a + pos + take] = w
            n_gw[g, w] = take
            pos += take
        assert pos == n

    # chunk template per (w, ti): measured per-core max, shared across cores
    nch_tb = (
        -(-n_gw.reshape(N_CORES, N_TILES, N_WIN).max(axis=0) // P)
    ).T.copy()                                # [5, 98]

    tb_gbase = np.zeros((N_WIN, N_TILES), np.int64)
    chunk_ti = []
    wbase = np.zeros(N_WIN, np.int64)
    wn = np.zeros(N_WIN, np.int64)
    gc = 0
    for w in range(N_WIN):
        gc = ((gc + CALL_CH - 1) // CALL_CH) * CALL_CH
        wbase[w] = gc
        for ti_ in range(N_TILES):
            tb_gbase[w, ti_] = gc
            gc += nch_tb[w, ti_]
            chunk_ti.extend([ti_] * int(nch_tb[w, ti_]))
        wn[w] = gc - wbase[w]
    ctot = ((gc + CALL_CH - 1) // CALL_CH) * CALL_CH

    # --- per-edge slot assignment ------------------------------------------
    core_s = core[order0]
    ti_s = ti[order0]
    dl_s = dl[order0]
    key = (core_s * N_WIN + bk_s) * N_TILES + ti_s
    order1 = np.argsort(key, kind="stable")
    key_s = key[order1]
    counts = np.bincount(key, minlength=N_CORES * N_WIN * N_TILES)
    starts = np.zeros(N_CORES * N_WIN * N_TILES + 1, np.int64)
    np.cumsum(counts, out=starts[1:])
    rank = np.arange(E, dtype=np.int64) - starts[key_s]
    core_f = core_s[order1]
    w_f = bk_s[order1].astype(np.int64)
    gpos = tb_gbase[w_f, ti_s[order1]] * P + rank
    sl = (s_s[order1] - np.asarray(WSTART, np.int64)[w_f]).astype(np.int16)

    idx_arr = np.zeros((N_CORES, 128, ctot * 8), np.int16)
    dl_arr = np.full((N_CORES, 128, ctot), 255.0, BF16)
    idx_arr[core_f, gpos % 16, gpos // 16] = sl
    dl_arr[core_f, gpos % 128, gpos // 128] = dl_s[order1].astype(BF16)

    ti_of_chunk = np.full(ctot, -1, np.int64)
    pos = 0
    for w in range(N_WIN):
        nb = int(wn[w])
        ti_of_chunk[int(wbase[w]) : int(wbase[w]) + nb] = chunk_ti[pos : pos + nb]
        pos += nb
    calls = []
    for w in range(N_WIN):
        nb = int(wn[w])
        for k in range((nb + CALL_CH - 1) // CALL_CH):
            c0 = int(wbase[w]) + CALL_CH * k
            nn = min(CALL_CH, nb - CALL_CH * k)
            calls.append((w, c0, nn, int(ti_of_chunk[c0])))
    # Q7 SWDGE reads the wrapped index block from each 16-partition group
    # (one per gpsimd core) -> replicate rows 0:16 into rows 16:128.
    idx_arr[:, 16:, :] = np.tile(idx_arr[:, :16, :], (1, 7, 1))

    meta = dict(nch_tb=nch_tb, tb_gbase=tb_gbase, calls=calls, ctot=ctot)
    return idx_arr, dl_arr, meta


def _build(ctx, tc, aps, metas):
    import concourse.mybir as mybir

    nc = tc.nc
    f32 = mybir.dt.float32
    bf16 = mybir.dt.bfloat16
    i16 = mybir.dt.int16
    Alu = mybir.AluOpType
    Act = mybir.ActivationFunctionType

    cp = ctx.enter_context(tc.tile_pool(name="const", bufs=1))

    def load(name, dtype):
        ap = aps[name].ap()
        t = cp.tile(list(ap.shape), dtype, tag=name)
        nc.sync.dma_start(out=t[:], in_=ap[:])
        return t

    # The first gathers must not wait on the full 2MB idx tensors (tile-
    # granular dependency tracking), so the head calls read from small
    # SEPARATE head tensors that load first.
    idxh_t = [load("idxh0", i16), load("idxh1", i16)]
    idx_t = [load("idx0", i16), load("idx1", i16)]
    dl_t = [load("dl0", bf16), load("dl1", bf16)]
    iota_t = load("iota8", bf16)
    ident_t = load("ident", bf16)
    wh_t = [load("wh0", bf16), load("wh1", bf16)]
    bh_t = [load("bh0", bf16), load("bh1", bf16)]
    u_t = [load("u0", bf16), load("u1", bf16)]
    dvh_t = [load("dvh0", f32), load("dvh1", f32)]

    xb_ap = [aps["xb0"].ap(), aps["xb1"].ap()]
    xs_ap = aps["xs"].ap()
    out_ap = aps["out"].ap()

    gp = ctx.enter_context(tc.tile_pool(name="g", bufs=20))
    s8p = ctx.enter_context(tc.tile_pool(name="s8", bufs=16))
    xlp = ctx.enter_context(tc.tile_pool(name="xl", bufs=4))
    aggp = ctx.enter_context(tc.tile_pool(name="agg", bufs=4))
    rp = ctx.enter_context(tc.tile_pool(name="r", bufs=4))
    op_ = ctx.enter_context(tc.tile_pool(name="o", bufs=3))
    ps_t = ctx.enter_context(tc.tile_pool(name="psT", bufs=4, space="PSUM"))
    ps_b = ctx.enter_context(tc.tile_pool(name="psB", bufs=2, space="PSUM"))

    # --- emit all gather calls in consumption order -------------------------
    all_calls = []
    for d in (0, 1):
        for (w, c0, nn, fti) in metas[d]["calls"]:
            all_calls.append((fti, d, w, c0, nn))
    all_calls.sort()

    # one shared register per distinct num_idxs value: a fresh to_reg per call
    # would put 250 MOVEs on the serial Pool stream (~15us)
    nregs = {}

    def nreg(n):
        r = nregs.get(n)
        if r is None:
            r = nc.gpsimd.to_reg(n)
            nregs[n] = r
        return r

    G = [{}, {}]
    qctr = 0
    for (fti, d, w, c0, nn) in all_calls:
        g = gp.tile([128, nn * 128], bf16, tag="g", name="g")
        src_t = idxh_t[d] if (c0 + nn) * 8 <= HEAD_COLS else idx_t[d]
        nc.gpsimd.dma_gather(
            out_ap=g[:].rearrange("p (c e) -> p c e", e=128),
            in_ap=xb_ap[d][WSTART[w] : WSTART[w] + WLEN, :],
            idxs_ap=src_t[:, c0 * 8 : (c0 + nn) * 8],
            num_idxs=nn * 128,
            num_idxs_reg=nreg(nn * 128),
            elem_size=128,
            queue_num=qctr % 4,
        )
        qctr += 1
        G[d][c0 // CALL_CH] = g

    # --- main tile loop -----------------------------------------------------
    S8 = [{}, {}]

    def get_s8(d, batch):
        t = S8[d].get(batch)
        if t is None:
            t = s8p.tile([128, 1024], bf16, tag="s8", name="s8")
            nc.vector.tensor_tensor(
                out=t[:].rearrange("p (c e) -> p c e", e=128),
                in0=iota_t[:].rearrange("p (c e) -> p c e", e=128),
                in1=dl_t[d][:, batch * 8 : batch * 8 + 8]
                .unsqueeze(2)
                .broadcast_to([128, 8, 128]),
                op=Alu.is_equal,
            )
            S8[d][batch] = t
        return t

    for ti in range(N_TILES):
        r_ = [None, None]
        xsl = xlp.tile([128, 2 * D], bf16, tag="xl")
        nc.sync.dma_start(out=xsl[:], in_=xs_ap[ti * P : (ti + 1) * P, :])
        for d in (0, 1):
            m = metas[d]
            total_ch = int(m["nch_tb"][:, ti].sum())
            psT = ps_t.tile([D, 128], f32, tag="psT")
            nc.tensor.matmul(
                out=psT[:], lhsT=xsl[:, d * D : (d + 1) * D], rhs=ident_t[:],
                start=True, stop=(total_ch == 0),
            )
            done = 0
            for w in range(N_WIN):
                n = int(m["nch_tb"][w, ti])
                base = int(m["tb_gbase"][w, ti])
                for cc in range(n):
                    gc = base + cc
                    s8 = get_s8(d, gc // 8)
                    g = G[d][gc // CALL_CH]
                    col = (gc % CALL_CH) * 128
                    scol = (gc % 8) * 128
                    done += 1
                    nc.tensor.matmul(
                        out=psT[:],
                        lhsT=g[:, col : col + D],
                        rhs=s8[:, scol : scol + 128],
                        start=False, stop=(done == total_ch),
                    )
            aggT = aggp.tile([D, 128], bf16, tag="agg")
            nc.scalar.activation(out=aggT[:], in_=psT[:], func=Act.Copy)
            psB = ps_b.tile([128, D], f32, tag="psB")
            nc.tensor.matmul(
                out=psB[:], lhsT=aggT[:], rhs=wh_t[d][:], start=True, stop=False
            )
            nc.tensor.matmul(
                out=psB[:],
                lhsT=u_t[d][:, ti * P : (ti + 1) * P],
                rhs=bh_t[d][:],
                start=False, stop=True,
            )
            r_[d] = rp.tile([128, D], f32, name=f"r{d}", tag=f"r{d}")
            nc.scalar.activation(
                out=r_[d][:], in_=psB[:], func=Act.Relu,
                scale=dvh_t[d][:, ti : ti + 1],
            )
        o = op_.tile([128, D], f32, tag="o")
        nc.vector.tensor_add(out=o[:], in0=r_[0][:], in1=r_[1][:])
        nc.sync.dma_start(
            out=out_ap[ti * P : (ti + 1) * P, :], in_=o[:, :]
        )


def kernel(x, edge_index, W_f, b_f, W_b, b_b):
    global LAST_RESULTS
    import concourse.tile as tile
    from concourse import bacc, mybir
    from concourse import bass_utils

    x = np.asarray(x, dtype=np.float32)
    ei = np.asarray(edge_index).astype(np.int64)
    W_f = np.asarray(W_f, dtype=np.float32)
    b_f = np.asarray(b_f, dtype=np.float32)
    W_b = np.asarray(W_b, dtype=np.float32)
    b_b = np.asarray(b_b, dtype=np.float32)
    src, dst = ei[0], ei[1]

    ideg_f = np.bincount(dst, minlength=N_NODES)
    ideg_b = np.bincount(src, minlength=N_NODES)
    deg_f = (ideg_f + 1).astype(np.float32)
    deg_b = (ideg_b + 1).astype(np.float32)
    dinv_f = (1.0 / np.sqrt(deg_f)).astype(np.float32)
    dinv_b = (1.0 / np.sqrt(deg_b)).astype(np.float32)
    dinvs = [dinv_f, dinv_b]
    degs = [deg_f, deg_b]

    # balanced node -> slot permutation (shared by both directions)
    slot = _balance_nodes(np.stack([ideg_f, ideg_b]))

    # direction 0 (forward): messages src -> dst; direction 1: dst -> src
    prep = [_prep_dir(slot[dst], src), _prep_dir(slot[src], dst)]
    metas = [prep[0][2], prep[1][2]]

    # pre-scaled gather sources x~ = dinv * x (bf16, padded to 128 cols)
    # and permuted per-slot arrays
    occupied = np.zeros(N_CORES * TILE_PAD, bool)
    occupied[slot] = True
    node_of_slot = np.zeros(N_CORES * TILE_PAD, np.int64)
    node_of_slot[slot] = np.arange(N_NODES)

    xb = []
    u_arr = []
    dvh = []
    xself = np.zeros((N_CORES, TILE_PAD, 2 * D), dtype=BF16)
    for d in (0, 1):
        xt = (x * dinvs[d][:, None]).astype(BF16)
        xbd = np.zeros((N_NODES, 128), dtype=BF16)
        xbd[:, :D] = xt
        xb.append(xbd)
        slot_dinv = np.where(occupied, dinvs[d][node_of_slot], 0.0).astype(np.float32)
        slot_u = np.where(occupied, np.sqrt(degs[d][node_of_slot]), 0.0)
        xs_flat = np.zeros((N_CORES * TILE_PAD, D), dtype=BF16)
        xs_flat[occupied] = xt[node_of_slot[occupied]]
        xself[:, :, d * D : (d + 1) * D] = xs_flat.reshape(N_CORES, TILE_PAD, D)
        u_arr.append(slot_u.reshape(N_CORES, 1, TILE_PAD).astype(BF16))
        dvh.append(
            (0.5 * slot_dinv).reshape(N_CORES, N_TILES, 128).transpose(0, 2, 1).copy()
        )

    iota8 = np.tile(np.arange(128, dtype=np.float32), 8).reshape(1, 1024)
    iota8 = np.broadcast_to(iota8, (128, 1024)).astype(BF16).copy()
    ident = np.eye(128, dtype=np.float32).astype(BF16)
    whs = [W_f.astype(BF16), W_b.astype(BF16)]
    bhs = [b_f.reshape(1, D).astype(BF16), b_b.reshape(1, D).astype(BF16)]

    nc = bacc.Bacc(
        "TRN2",
        target_bir_lowering=False,
        debug=False,
        enable_asserts=False,
        num_devices=N_CORES,
        num_swdge_queues=4,
        dynamic_dma_scratch_size=49152,
    )
    dt = mybir.dt
    aps = {}
    aps["iota8"] = nc.dram_tensor("iota8", [128, 1024], dt.bfloat16, kind="ExternalInput")
    aps["ident"] = nc.dram_tensor("ident", [128, 128], dt.bfloat16, kind="ExternalInput")
    aps["xs"] = nc.dram_tensor("xs", [TILE_PAD, 2 * D], dt.bfloat16, kind="ExternalInput")
    for d in (0, 1):
        ct = metas[d]["ctot"]
        aps[f"xb{d}"] = nc.dram_tensor(f"xb{d}", [N_NODES, 128], dt.bfloat16, kind="ExternalInput")
        aps[f"wh{d}"] = nc.dram_tensor(f"wh{d}", [D, D], dt.bfloat16, kind="ExternalInput")
        aps[f"bh{d}"] = nc.dram_tensor(f"bh{d}", [1, D], dt.bfloat16, kind="ExternalInput")
        aps[f"u{d}"] = nc.dram_tensor(f"u{d}", [1, TILE_PAD], dt.bfloat16, kind="ExternalInput")
        aps[f"dvh{d}"] = nc.dram_tensor(f"dvh{d}", [128, N_TILES], dt.float32, kind="ExternalInput")
        aps[f"idx{d}"] = nc.dram_tensor(f"idx{d}", [128, ct * 8], dt.int16, kind="ExternalInput")
        aps[f"idxh{d}"] = nc.dram_tensor(f"idxh{d}", [128, HEAD_COLS], dt.int16, kind="ExternalInput")
        aps[f"dl{d}"] = nc.dram_tensor(f"dl{d}", [128, ct], dt.bfloat16, kind="ExternalInput")
    aps["out"] = nc.dram_tensor("out", [TILE_PAD, D], dt.float32, kind="ExternalOutput")

    with tile.TileContext(nc) as tc, ExitStack() as ctx:
        _build(ctx, tc, aps, metas)
    nc.compile()

    in_maps = []
    for c in range(N_CORES):
        m = {"iota8": iota8, "ident": ident, "xs": xself[c]}
        for d in (0, 1):
            idx_arr, dl_arr, _ = prep[d]
            m[f"xb{d}"] = xb[d]
            m[f"wh{d}"] = whs[d]
            m[f"bh{d}"] = bhs[d]
            m[f"u{d}"] = u_arr[d][c]
            m[f"dvh{d}"] = dvh[d][c]
            m[f"idx{d}"] = idx_arr[c]
            m[f"idxh{d}"] = np.ascontiguousarray(idx_arr[c][:, :HEAD_COLS])
            m[f"dl{d}"] = dl_arr[c]
        in_maps.append(m)

    LAST_RESULTS = bass_utils.run_bass_kernel_spmd(
        nc, in_maps, core_ids=list(range(N_CORES))
    )
    allout = np.concatenate([r["out"] for r in LAST_RESULTS.results], axis=0)
    return allout[slot].astype(np.float32)


# revision 22
# speedup vs baseline: 1.5204x; 1.0041x over previous
"""DirectedGCNConv on 8 Trainium2 NeuronCores (Bass/Tile).

Strategy: target nodes sharded across the 8 cores, edges partitioned by
target, 64x64 weights replicated.  The symmetric norm FACTORIZES:
norm_e = dinv[s]*dinv[t], so the kernel gathers from host-prescaled
x~ = dinv * x, accumulates with a pure 0/1 one-hot scatter matmul, and
applies dinv[t] (with the final 0.5 folded in) as the per-partition scale of
the output relu.  Bias enters via a rank-1 matmul with u = sqrt(deg).

Load balancing: dst nodes are assigned to the 784 (core, tile) bins by a
capacity-constrained 2D LPT on (in-deg_fwd, in-deg_bwd) so every tile sees
~E/784 edges in BOTH directions (the bass program is shared SPMD, so chunk
counts take the max over cores -- balancing kills that padding).  Sources are
split over 5 OVERLAPPING 32768-row windows (int16 gather indices); each edge
picks a covering window greedily so windows 0..3 fill to exactly cap=2 chunks
(256 edges, zero pad) and window 4 takes the remainder.  The host unpermutes
the output rows at the end.

Device-side per core, per direction:
  - x~ rows (bf16, padded to 128 cols = 256B) fetched with dma_gather in
    1024-index calls (the HW max), round-robin over the 4 SWDGE queues.
  - the 0/1 one-hot S is built 8 chunks at a time with ONE DVE tensor_tensor
    is_equal op (iota pattern vs dl broadcast along the free dim).
  - per 128-edge chunk one TensorE matmul accumulates aggT[64f, 128d] in PSUM;
    the self loop is an identity-matmul of the (permuted, host-gathered) x~
    slice; aggT -> SBUF bf16 via ACT copy, W-matmul + bias matmul, relu with
    scale=0.5*dinv on ACT; directions summed on DVE, written out.
"""

import heapq
from contextlib import ExitStack

import ml_dtypes
import numpy as np

N_NODES = 100000
D = 64
N_CORES = 8
RPC = N_NODES // N_CORES          # 12500 target rows per core
P = 128
N_TILES = (RPC + P - 1) // P      # 98
TILE_PAD = N_TILES * P            # 12544
N_BINS = N_CORES * N_TILES        # 784 (every tile is fully used; 12500*8 = 98*128*8 - pad)
WLEN = 32768
WSTART = [0, 16808, 33616, 50424, 67232]
N_WIN = 5
CALL_CH = 8                       # chunks per dma_gather call (8*128 = 1024 idx, HW max)
HEAD_COLS = 1536                  # idx head tensor cols (first ~24 calls/dir)

BF16 = ml_dtypes.bfloat16
LAST_RESULTS = None


def _balance_nodes(degs):
    """Assign nodes to N_BINS bins of <=128 nodes so that BOTH per-direction
    degree sums stay at/below the 10-chunk boundary (1280).  Pair nodes with
    opposite deg_f - deg_b residuals (each pair ~balanced across directions),
    LPT the pairs on their total, then swap-repair bins over the cap.
    Returns slot[node] in [0, N_CORES*TILE_PAD)."""
    df = degs[0].astype(np.int64)
    db = degs[1].astype(np.int64)
    order = np.argsort(df - db, kind="stable")
    half = N_NODES // 2
    pa, pb = order[:half], order[N_NODES - half :][::-1]   # opposite residuals
    ptot = df[pa] + db[pa] + df[pb] + db[pb]

    porder = np.argsort(-ptot, kind="stable")
    heap = [(0, i) for i in range(N_BINS)]
    heapq.heapify(heap)
    counts = np.zeros(N_BINS, np.int64)
    binof = np.empty(N_NODES, np.int64)
    pair_cap = 64                                          # 128 nodes per bin
    for pi in porder:
        while True:
            load, i = heapq.heappop(heap)
            if counts[i] < pair_cap:
                break
        binof[pa[pi]] = i
        binof[pb[pi]] = i
        counts[i] += 1
        if counts[i] < pair_cap:
            heapq.heappush(heap, (load + int(ptot[pi]), i))

    # swap-repair: force lf <= CAP and lb <= CAP where possible
    CAP = 1280
    lf = np.bincount(binof, weights=df, minlength=N_BINS).astype(np.int64)
    lb = np.bincount(binof, weights=db, minlength=N_BINS).astype(np.int64)
    members = [[] for _ in range(N_BINS)]
    for n in range(N_NODES):
        members[binof[n]].append(n)
    for _ in range(4):
        viol = [i for i in range(N_BINS) if lf[i] > CAP or lb[i] > CAP]
        if not viol:
            break
        slack_bins = sorted(
            (i for i in range(N_BINS) if lf[i] < CAP - 2 and lb[i] < CAP - 2),
            key=lambda i: lf[i] + lb[i],
        )
        for i in viol:
            guard = 0
            while (lf[i] > CAP or lb[i] > CAP) and guard < 40:
                guard += 1
                use_f = lf[i] - CAP >= lb[i] - CAP
                mem = members[i]
                n_out = max(mem, key=(lambda n: df[n]) if use_f else (lambda n: db[n]))
                swapped = False
                for j in slack_bins:
                    if j == i:
                        continue
                    m_in = min(members[j], key=lambda n: df[n] + db[n])
                    nlf_j = lf[j] + df[n_out] - df[m_in]
                    nlb_j = lb[j] + db[n_out] - db[m_in]
                    if nlf_j > CAP or nlb_j > CAP:
                        continue
                    if df[m_in] >= df[n_out] and db[m_in] >= db[n_out]:
                        continue
                    members[i].remove(n_out)
                    members[j].remove(m_in)
                    members[i].append(m_in)
                    members[j].append(n_out)
                    lf[i] += df[m_in] - df[n_out]
                    lb[i] += db[m_in] - db[n_out]
                    lf[j] = nlf_j
                    lb[j] = nlb_j
                    binof[n_out] = j
                    binof[m_in] = i
                    swapped = True
                    break
                if not swapped:
                    break

    # slot within bin: arbitrary order
    slot = np.empty(N_NODES, np.int64)
    offs = np.zeros(N_BINS, np.int64)
    for n in range(N_NODES):
        i = binof[n]
        core, ti = i // N_TILES, i % N_TILES
        slot[n] = core * TILE_PAD + ti * P + offs[i]
        offs[i] += 1
    return slot


def _prep_dir(tslot, s):
    """Host-side edge partitioning for one direction.

    tslot = target slot (already permuted, in [0, N_CORES*TILE_PAD));
    s = source node id.  Window-major chunk layout, CALL_CH-aligned window
    bases."""
    E = tslot.shape[0]
    core = tslot // TILE_PAD
    tl = tslot - core * TILE_PAD
    ti = tl // P
    dl = tl - ti * P

    # --- greedy window assignment with per-(ti) caps ------------------------
    # caps: windows 0..3 take exactly 2 chunks (256), window 4 the rest.
    grp = (core * N_TILES + ti)
    order0 = np.argsort(grp * np.int64(N_NODES) + s, kind="stable")
    grp_s = grp[order0]
    s_s = s[order0]
    gcounts = np.bincount(grp, minlength=N_CORES * N_TILES)
    gstart = np.zeros(N_CORES * N_TILES + 1, np.int64)
    np.cumsum(gcounts, out=gstart[1:])

    # cumulative mandatory counts: edges with src < WSTART[w+1] must be
    # assigned to windows <= w.  Template cumulative caps (shared across
    # cores) = max over cores, rounded up to whole chunks, floor 2 chunks per
    # window.
    cum_mand = np.zeros((N_CORES * N_TILES, N_WIN), np.int64)
    for g in range(N_CORES * N_TILES):
        a, e = gstart[g], gstart[g + 1]
        src = s_s[a:e]
        for w in range(N_WIN - 1):
            cum_mand[g, w] = np.searchsorted(src, WSTART[w + 1])
        cum_mand[g, N_WIN - 1] = e - a
    cm = cum_mand.reshape(N_CORES, N_TILES, N_WIN).max(axis=0)   # [98, 5]
    cumcap = -(-cm // P) * P
    for w in range(N_WIN):
        cumcap[:, w] = np.maximum(cumcap[:, w], 2 * P * (w + 1))
    for w in range(1, N_WIN):
        cumcap[:, w] = np.maximum(cumcap[:, w], cumcap[:, w - 1] + P)
    caps_ti = np.empty((N_TILES, N_WIN), np.int64)
    caps_ti[:, 0] = cumcap[:, 0]
    caps_ti[:, 1:] = cumcap[:, 1:] - cumcap[:, :-1]
    caps_ti[:, N_WIN - 1] = 1 << 30           # last window absorbs any spill

    bk_s = np.empty(E, np.int8)
    n_gw = np.zeros((N_CORES * N_TILES, N_WIN), np.int64)
    for g in range(N_CORES * N_TILES):
        a, e = gstart[g], gstart[g + 1]
        src = s_s[a:e]                       # sorted ascending within group
        caps = caps_ti[g % N_TILES]
        pos = 0
        n = e - a
        for w in range(N_WIN):
            hi = np.searchsorted(src, WSTART[w] + WLEN)
            take = min(int(caps[w]), hi - pos)
            if w + 1 < N_WIN:
                mand = np.searchsorted(src, WSTART[w + 1]) - pos
                assert mand <= caps[w], (g, w, mand, caps[w])
            else:
                take = n - pos
            bk_s[a + pos : a + pos + take] = w
            n_gw[g, w] = take
            pos += take
        assert pos == n

    # chunk template per (w, ti): measured per-core max, shared across cores
    nch_tb = (
        -(-n_gw.reshape(N_CORES, N_TILES, N_WIN).max(axis=0) // P)
    ).T.copy()                                # [5, 98]

    tb_gbase = np.zeros((N_WIN, N_TILES), np.int64)
    chunk_ti = []
    wbase = np.zeros(N_WIN, np.int64)
    wn = np.zeros(N_WIN, np.int64)
    gc = 0
    for w in range(N_WIN):
        gc = ((gc + CALL_CH - 1) // CALL_CH) * CALL_CH
        wbase[w] = gc
        for ti_ in range(N_TILES):
            tb_gbase[w, ti_] = gc
            gc += nch_tb[w, ti_]
            chunk_ti.extend([ti_] * int(nch_tb[w, ti_]))
        wn[w] = gc - wbase[w]
    ctot = ((gc + CALL_CH - 1) // CALL_CH) * CALL_CH

    # --- per-edge slot assignment ------------------------------------------
    core_s = core[order0]
    ti_s = ti[order0]
    dl_s = dl[order0]
    key = (core_s * N_WIN + bk_s) * N_TILES + ti_s
    order1 = np.argsort(key, kind="stable")
    key_s = key[order1]
    counts = np.bincount(key, minlength=N_CORES * N_WIN * N_TILES)
    starts = np.zeros(N_CORES * N_WIN * N_TILES + 1, np.int64)
    np.cumsum(counts, out=starts[1:])
    rank = np.arange(E, dtype=np.int64) - starts[key_s]
    core_f = core_s[order1]
    w_f = bk_s[order1].astype(np.int64)
    gpos = tb_gbase[w_f, ti_s[order1]] * P + rank
    sl = (s_s[order1] - np.asarray(WSTART, np.int64)[w_f]).astype(np.int16)

    idx_arr = np.zeros((N_CORES, 128, ctot * 8), np.int16)
    dl_arr = np.full((N_CORES, 128, ctot), 255.0, BF16)
    idx_arr[core_f, gpos % 16, gpos // 16] = sl
    dl_arr[core_f, gpos % 128, gpos // 128] = dl_s[order1].astype(BF16)

    ti_of_chunk = np.full(ctot, -1, np.int64)
    pos = 0
    for w in range(N_WIN):
        nb = int(wn[w])
        ti_of_chunk[int(wbase[w]) : int(wbase[w]) + nb] = chunk_ti[pos : pos + nb]
        pos += nb
    calls = []
    for w in range(N_WIN):
        nb = int(wn[w])
        for k in range((nb + CALL_CH - 1) // CALL_CH):
            c0 = int(wbase[w]) + CALL_CH * k
            nn = min(CALL_CH, nb - CALL_CH * k)
            calls.append((w, c0, nn, int(ti_of_chunk[c0])))
    # Q7 SWDGE reads the wrapped index block from each 16-partition group
    # (one per gpsimd core) -> replicate rows 0:16 into rows 16:128.
    idx_arr[:, 16:, :] = np.tile(idx_arr[:, :16, :], (1, 7, 1))

    meta = dict(nch_tb=nch_tb, tb_gbase=tb_gbase, calls=calls, ctot=ctot)
    return idx_arr, dl_arr, meta


def _build(ctx, tc, aps, metas):
    import concourse.mybir as mybir

    nc = tc.nc
    f32 = mybir.dt.float32
    bf16 = mybir.dt.bfloat16
    i16 = mybir.dt.int16
    Alu = mybir.AluOpType
    Act = mybir.ActivationFunctionType

    cp = ctx.enter_context(tc.tile_pool(name="const", bufs=1))

    def load(name, dtype):
        ap = aps[name].ap()
        t = cp.tile(list(ap.shape), dtype, tag=name)
        nc.sync.dma_start(out=t[:], in_=ap[:])
        return t

    # The first gathers must not wait on the full 2MB idx tensors (tile-
    # granular dependency tracking), so the head calls read from small
    # SEPARATE head tensors that load first.
    widx_t = load("widx", i16)
    idxh_t = [load("idxh0", i16), load("idxh1", i16)]
    idx_t = [load("idx0", i16), load("idx1", i16)]
    dl_t = [load("dl0", bf16), load("dl1", bf16)]
    iota_t = load("iota8", bf16)
    ident_t = load("ident", bf16)
    wh_t = [load("wh0", bf16), load("wh1", bf16)]
    bh_t = [load("bh0", bf16), load("bh1", bf16)]
    u_t = [load("u0", bf16), load("u1", bf16)]
    dvh_t = [load("dvh0", f32), load("dvh1", f32)]

    xb_ap = [aps["xb0"].ap(), aps["xb1"].ap()]
    xs_ap = aps["xs"].ap()
    out_ap = aps["out"].ap()

    gp = ctx.enter_context(tc.tile_pool(name="g", bufs=20))
    s8p = ctx.enter_context(tc.tile_pool(name="s8", bufs=16))
    xlp = ctx.enter_context(tc.tile_pool(name="xl", bufs=4))
    aggp = ctx.enter_context(tc.tile_pool(name="agg", bufs=4))
    rp = ctx.enter_context(tc.tile_pool(name="r", bufs=4))
    op_ = ctx.enter_context(tc.tile_pool(name="o", bufs=3))
    ps_t = ctx.enter_context(tc.tile_pool(name="psT", bufs=4, space="PSUM"))
    ps_b = ctx.enter_context(tc.tile_pool(name="psB", bufs=2, space="PSUM"))

    # --- emit all gather calls in consumption order -------------------------
    all_calls = []
    for d in (0, 1):
        for (w, c0, nn, fti) in metas[d]["calls"]:
            all_calls.append((fti, d, w, c0, nn))
    all_calls.sort()

    # one shared register per distinct num_idxs value: a fresh to_reg per call
    # would put 250 MOVEs on the serial Pool stream (~15us)
    nregs = {}

    def nreg(n):
        r = nregs.get(n)
        if r is None:
            r = nc.gpsimd.to_reg(n)
            nregs[n] = r
        return r

    # one dummy 128-idx gather per SWDGE queue warms the Q7 gather-kernel
    # icache during the idle window while the real idx tensors stream in
    # (the first real call per queue otherwise pays ~10us of cold start).
    wp = ctx.enter_context(tc.tile_pool(name="wu", bufs=1))
    wt = wp.tile([128, 4 * 128], bf16, tag="wu")
    for q in range(4):
        nc.gpsimd.dma_gather(
            out_ap=wt[:, q * 128 : (q + 1) * 128].rearrange(
                "p (c e) -> p c e", e=128
            ),
            in_ap=xb_ap[0][0:WLEN, :],
            idxs_ap=widx_t[:, 0:8],
            num_idxs=128,
            num_idxs_reg=nreg(128),
            elem_size=128,
            queue_num=q,
        )

    G = [{}, {}]
    qctr = 0
    for (fti, d, w, c0, nn) in all_calls:
        g = gp.tile([128, nn * 128], bf16, tag="g", name="g")
        src_t = idxh_t[d] if (c0 + nn) * 8 <= HEAD_COLS else idx_t[d]
        nc.gpsimd.dma_gather(
            out_ap=g[:].rearrange("p (c e) -> p c e", e=128),
            in_ap=xb_ap[d][WSTART[w] : WSTART[w] + WLEN, :],
            idxs_ap=src_t[:, c0 * 8 : (c0 + nn) * 8],
            num_idxs=nn * 128,
            num_idxs_reg=nreg(nn * 128),
            elem_size=128,
            queue_num=qctr % 4,
        )
        qctr += 1
        G[d][c0 // CALL_CH] = g

    # --- main tile loop -----------------------------------------------------
    S8 = [{}, {}]

    def get_s8(d, batch):
        t = S8[d].get(batch)
        if t is None:
            t = s8p.tile([128, 1024], bf16, tag="s8", name="s8")
            nc.vector.tensor_tensor(
                out=t[:].rearrange("p (c e) -> p c e", e=128),
                in0=iota_t[:].rearrange("p (c e) -> p c e", e=128),
                in1=dl_t[d][:, batch * 8 : batch * 8 + 8]
                .unsqueeze(2)
                .broadcast_to([128, 8, 128]),
                op=Alu.is_equal,
            )
            S8[d][batch] = t
        return t

    for ti in range(N_TILES):
        r_ = [None, None]
        xsl = xlp.tile([128, 2 * D], bf16, tag="xl")
        nc.sync.dma_start(out=xsl[:], in_=xs_ap[ti * P : (ti + 1) * P, :])
        for d in (0, 1):
            m = metas[d]
            total_ch = int(m["nch_tb"][:, ti].sum())
            psT = ps_t.tile([D, 128], f32, tag="psT")
            nc.tensor.matmul(
                out=psT[:], lhsT=xsl[:, d * D : (d + 1) * D], rhs=ident_t[:],
                start=True, stop=(total_ch == 0),
            )
            done = 0
            for w in range(N_WIN):
                n = int(m["nch_tb"][w, ti])
                base = int(m["tb_gbase"][w, ti])
                for cc in range(n):
                    gc = base + cc
                    s8 = get_s8(d, gc // 8)
                    g = G[d][gc // CALL_CH]
                    col = (gc % CALL_CH) * 128
                    scol = (gc % 8) * 128
                    done += 1
                    nc.tensor.matmul(
                        out=psT[:],
                        lhsT=g[:, col : col + D],
                        rhs=s8[:, scol : scol + 128],
                        start=False, stop=(done == total_ch),
                    )
            aggT = aggp.tile([D, 128], bf16, tag="agg")
            nc.scalar.activation(out=aggT[:], in_=psT[:], func=Act.Copy)
            psB = ps_b.tile([128, D], f32, tag="psB")
            nc.tensor.matmul(
                out=psB[:], lhsT=aggT[:], rhs=wh_t[d][:], start=True, stop=False
            )
            nc.tensor.matmul(
                out=psB[:],
                lhsT=u_t[d][:, ti * P : (ti + 1) * P],
                rhs=bh_t[d][:],
                start=False, stop=True,
            )
            r_[d] = rp.tile([128, D], f32, name=f"r{d}", tag=f"r{d}")
            nc.scalar.activation(
                out=r_[d][:], in_=psB[:], func=Act.Relu,
                scale=dvh_t[d][:, ti : ti + 1],
            )
        o = op_.tile([128, D], f32, tag="o")
        nc.vector.tensor_add(out=o[:], in0=r_[0][:], in1=r_[1][:])
        nc.sync.dma_start(
            out=out_ap[ti * P : (ti + 1) * P, :], in_=o[:, :]
        )


def kernel(x, edge_index, W_f, b_f, W_b, b_b):
    global LAST_RESULTS
    import concourse.tile as tile
    from concourse import bacc, mybir
    from concourse import bass_utils

    x = np.asarray(x, dtype=np.float32)
    ei = np.asarray(edge_index).astype(np.int64)
    W_f = np.asarray(W_f, dtype=np.float32)
    b_f = np.asarray(b_f, dtype=np.float32)
    W_b = np.asarray(W_b, dtype=np.float32)
    b_b = np.asarray(b_b, dtype=np.float32)
    src, dst = ei[0], ei[1]

    ideg_f = np.bincount(dst, minlength=N_NODES)
    ideg_b = np.bincount(src, minlength=N_NODES)
    deg_f = (ideg_f + 1).astype(np.float32)
    deg_b = (ideg_b + 1).astype(np.float32)
    dinv_f = (1.0 / np.sqrt(deg_f)).astype(np.float32)
    dinv_b = (1.0 / np.sqrt(deg_b)).astype(np.float32)
    dinvs = [dinv_f, dinv_b]
    degs = [deg_f, deg_b]

    # balanced node -> slot permutation (shared by both directions)
    slot = _balance_nodes(np.stack([ideg_f, ideg_b]))

    # direction 0 (forward): messages src -> dst; direction 1: dst -> src
    prep = [_prep_dir(slot[dst], src), _prep_dir(slot[src], dst)]
    metas = [prep[0][2], prep[1][2]]

    # pre-scaled gather sources x~ = dinv * x (bf16, padded to 128 cols)
    # and permuted per-slot arrays
    occupied = np.zeros(N_CORES * TILE_PAD, bool)
    occupied[slot] = True
    node_of_slot = np.zeros(N_CORES * TILE_PAD, np.int64)
    node_of_slot[slot] = np.arange(N_NODES)

    xb = []
    u_arr = []
    dvh = []
    xself = np.zeros((N_CORES, TILE_PAD, 2 * D), dtype=BF16)
    for d in (0, 1):
        xt = (x * dinvs[d][:, None]).astype(BF16)
        xbd = np.zeros((N_NODES, 128), dtype=BF16)
        xbd[:, :D] = xt
        xb.append(xbd)
        slot_dinv = np.where(occupied, dinvs[d][node_of_slot], 0.0).astype(np.float32)
        slot_u = np.where(occupied, np.sqrt(degs[d][node_of_slot]), 0.0)
        xs_flat = np.zeros((N_CORES * TILE_PAD, D), dtype=BF16)
        xs_flat[occupied] = xt[node_of_slot[occupied]]
        xself[:, :, d * D : (d + 1) * D] = xs_flat.reshape(N_CORES, TILE_PAD, D)
        u_arr.append(slot_u.reshape(N_CORES, 1, TILE_PAD).astype(BF16))
        dvh.append(
            (0.5 * slot_dinv).reshape(N_CORES, N_TILES, 128).transpose(0, 2, 1).copy()
        )

    iota8 = np.tile(np.arange(128, dtype=np.float32), 8).reshape(1, 1024)
    iota8 = np.broadcast_to(iota8, (128, 1024)).astype(BF16).copy()
    ident = np.eye(128, dtype=np.float32).astype(BF16)
    whs = [W_f.astype(BF16), W_b.astype(BF16)]
    bhs = [b_f.reshape(1, D).astype(BF16), b_b.reshape(1, D).astype(BF16)]

    nc = bacc.Bacc(
        "TRN2",
        target_bir_lowering=False,
        debug=False,
        enable_asserts=False,
        num_devices=N_CORES,
        num_swdge_queues=4,
        dynamic_dma_scratch_size=49152,
    )
    dt = mybir.dt
    aps = {}
    aps["iota8"] = nc.dram_tensor("iota8", [128, 1024], dt.bfloat16, kind="ExternalInput")
    aps["widx"] = nc.dram_tensor("widx", [128, 16], dt.int16, kind="ExternalInput")
    aps["ident"] = nc.dram_tensor("ident", [128, 128], dt.bfloat16, kind="ExternalInput")
    aps["xs"] = nc.dram_tensor("xs", [TILE_PAD, 2 * D], dt.bfloat16, kind="ExternalInput")
    for d in (0, 1):
        ct = metas[d]["ctot"]
        aps[f"xb{d}"] = nc.dram_tensor(f"xb{d}", [N_NODES, 128], dt.bfloat16, kind="ExternalInput")
        aps[f"wh{d}"] = nc.dram_tensor(f"wh{d}", [D, D], dt.bfloat16, kind="ExternalInput")
        aps[f"bh{d}"] = nc.dram_tensor(f"bh{d}", [1, D], dt.bfloat16, kind="ExternalInput")
        aps[f"u{d}"] = nc.dram_tensor(f"u{d}", [1, TILE_PAD], dt.bfloat16, kind="ExternalInput")
        aps[f"dvh{d}"] = nc.dram_tensor(f"dvh{d}", [128, N_TILES], dt.float32, kind="ExternalInput")
        aps[f"idx{d}"] = nc.dram_tensor(f"idx{d}", [128, ct * 8], dt.int16, kind="ExternalInput")
        aps[f"idxh{d}"] = nc.dram_tensor(f"idxh{d}", [128, HEAD_COLS], dt.int16, kind="ExternalInput")
        aps[f"dl{d}"] = nc.dram_tensor(f"dl{d}", [128, ct], dt.bfloat16, kind="ExternalInput")
    aps["out"] = nc.dram_tensor("out", [TILE_PAD, D], dt.float32, kind="ExternalOutput")

    with tile.TileContext(nc) as tc, ExitStack() as ctx:
        _build(ctx, tc, aps, metas)
    nc.compile()

    in_maps = []
    for c in range(N_CORES):
        m = {"iota8": iota8, "ident": ident, "xs": xself[c],
             "widx": np.zeros((128, 16), np.int16)}
        for d in (0, 1):
            idx_arr, dl_arr, _ = prep[d]
            m[f"xb{d}"] = xb[d]
            m[f"wh{d}"] = whs[d]
            m[f"bh{d}"] = bhs[d]
            m[f"u{d}"] = u_arr[d][c]
            m[f"dvh{d}"] = dvh[d][c]
            m[f"idx{d}"] = idx_arr[c]
            m[f"idxh{d}"] = np.ascontiguousarray(idx_arr[c][:, :HEAD_COLS])
            m[f"dl{d}"] = dl_arr[c]
        in_maps.append(m)

    LAST_RESULTS = bass_utils.run_bass_kernel_spmd(
        nc, in_maps, core_ids=list(range(N_CORES))
    )
    allout = np.concatenate([r["out"] for r in LAST_RESULTS.results], axis=0)
    return allout[slot].astype(np.float32)


# revision 25
# speedup vs baseline: 1.5205x; 1.0000x over previous
"""DirectedGCNConv on 8 Trainium2 NeuronCores (Bass/Tile).

Strategy: target nodes sharded across the 8 cores, edges partitioned by
target, 64x64 weights replicated.  The symmetric norm FACTORIZES:
norm_e = dinv[s]*dinv[t], so the kernel gathers from host-prescaled
x~ = dinv * x, accumulates with a pure 0/1 one-hot scatter matmul, and
applies dinv[t] (with the final 0.5 folded in) as the per-partition scale of
the output relu.  Bias enters via a rank-1 matmul with u = sqrt(deg).

Load balancing: dst nodes are assigned to the 784 (core, tile) bins by a
capacity-constrained 2D LPT on (in-deg_fwd, in-deg_bwd) so every tile sees
~E/784 edges in BOTH directions (the bass program is shared SPMD, so chunk
counts take the max over cores -- balancing kills that padding).  Sources are
split over 5 OVERLAPPING 32768-row windows (int16 gather indices); each edge
picks a covering window greedily so windows 0..3 fill to exactly cap=2 chunks
(256 edges, zero pad) and window 4 takes the remainder.  The host unpermutes
the output rows at the end.

Device-side per core, per direction:
  - x~ rows (bf16, padded to 128 cols = 256B) fetched with dma_gather in
    1024-index calls (the HW max), round-robin over the 4 SWDGE queues.
  - the 0/1 one-hot S is built 8 chunks at a time with ONE DVE tensor_tensor
    is_equal op (iota pattern vs dl broadcast along the free dim).
  - per 128-edge chunk one TensorE matmul accumulates aggT[64f, 128d] in PSUM;
    the self loop is an identity-matmul of the (permuted, host-gathered) x~
    slice; aggT -> SBUF bf16 via ACT copy, W-matmul + bias matmul, relu with
    scale=0.5*dinv on ACT; directions summed on DVE, written out.
"""

import heapq
from contextlib import ExitStack

import ml_dtypes
import numpy as np

N_NODES = 100000
D = 64
N_CORES = 8
RPC = N_NODES // N_CORES          # 12500 target rows per core
P = 128
N_TILES = (RPC + P - 1) // P      # 98
TILE_PAD = N_TILES * P            # 12544
N_BINS = N_CORES * N_TILES        # 784 (every tile is fully used; 12500*8 = 98*128*8 - pad)
WLEN = 32768
WSTART = [0, 16808, 33616, 50424, 67232]
N_WIN = 5
CALL_CH = 8                       # chunks per dma_gather call (8*128 = 1024 idx, HW max)
HEAD_COLS = 1536                  # idx head tensor cols (first ~24 calls/dir)

BF16 = ml_dtypes.bfloat16
LAST_RESULTS = None


def _balance_nodes(degs):
    """Assign nodes to N_BINS bins of <=128 nodes so that BOTH per-direction
    degree sums stay at/below the 10-chunk boundary (1280).  Pair nodes with
    opposite deg_f - deg_b residuals (each pair ~balanced across directions),
    LPT the pairs on their total, then swap-repair bins over the cap.
    Returns slot[node] in [0, N_CORES*TILE_PAD)."""
    df = degs[0].astype(np.int64)
    db = degs[1].astype(np.int64)
    order = np.argsort(df - db, kind="stable")
    half = N_NODES // 2
    pa, pb = order[:half], order[N_NODES - half :][::-1]   # opposite residuals
    ptot = df[pa] + db[pa] + df[pb] + db[pb]

    porder = np.argsort(-ptot, kind="stable")
    heap = [(0, i) for i in range(N_BINS)]
    heapq.heapify(heap)
    counts = np.zeros(N_BINS, np.int64)
    binof = np.empty(N_NODES, np.int64)
    pair_cap = 64                                          # 128 nodes per bin
    for pi in porder:
        while True:
            load, i = heapq.heappop(heap)
            if counts[i] < pair_cap:
                break
        binof[pa[pi]] = i
        binof[pb[pi]] = i
        counts[i] += 1
        if counts[i] < pair_cap:
            heapq.heappush(heap, (load + int(ptot[pi]), i))

    # swap-repair: force lf <= CAP and lb <= CAP where possible
    CAP = 1280
    lf = np.bincount(binof, weights=df, minlength=N_BINS).astype(np.int64)
    lb = np.bincount(binof, weights=db, minlength=N_BINS).astype(np.int64)
    members = [[] for _ in range(N_BINS)]
    for n in range(N_NODES):
        members[binof[n]].append(n)
    for _ in range(4):
        viol = [i for i in range(N_BINS) if lf[i] > CAP or lb[i] > CAP]
        if not viol:
            break
        slack_bins = sorted(
            (i for i in range(N_BINS) if lf[i] < CAP - 2 and lb[i] < CAP - 2),
            key=lambda i: lf[i] + lb[i],
        )
        for i in viol:
            guard = 0
            while (lf[i] > CAP or lb[i] > CAP) and guard < 40:
                guard += 1
                use_f = lf[i] - CAP >= lb[i] - CAP
                mem = members[i]
                n_out = max(mem, key=(lambda n: df[n]) if use_f else (lambda n: db[n]))
                swapped = False
                for j in slack_bins:
                    if j == i:
                        continue
                    m_in = min(members[j], key=lambda n: df[n] + db[n])
                    nlf_j = lf[j] + df[n_out] - df[m_in]
                    nlb_j = lb[j] + db[n_out] - db[m_in]
                    if nlf_j > CAP or nlb_j > CAP:
                        continue
                    if df[m_in] >= df[n_out] and db[m_in] >= db[n_out]:
                        continue
                    members[i].remove(n_out)
                    members[j].remove(m_in)
                    members[i].append(m_in)
                    members[j].append(n_out)
                    lf[i] += df[m_in] - df[n_out]
                    lb[i] += db[m_in] - db[n_out]
                    lf[j] = nlf_j
                    lb[j] = nlb_j
                    binof[n_out] = j
                    binof[m_in] = i
                    swapped = True
                    break
                if not swapped:
                    break

    # slot within bin: arbitrary order
    slot = np.empty(N_NODES, np.int64)
    offs = np.zeros(N_BINS, np.int64)
    for n in range(N_NODES):
        i = binof[n]
        core, ti = i // N_TILES, i % N_TILES
        slot[n] = core * TILE_PAD + ti * P + offs[i]
        offs[i] += 1
    return slot


def _prep_dir(tslot, s):
    """Host-side edge partitioning for one direction.

    tslot = target slot (already permuted, in [0, N_CORES*TILE_PAD));
    s = source node id.  Window-major chunk layout, CALL_CH-aligned window
    bases."""
    E = tslot.shape[0]
    core = tslot // TILE_PAD
    tl = tslot - core * TILE_PAD
    ti = tl // P
    dl = tl - ti * P

    # --- greedy window assignment with per-(ti) caps ------------------------
    # caps: windows 0..3 take exactly 2 chunks (256), window 4 the rest.
    grp = (core * N_TILES + ti)
    order0 = np.argsort(grp * np.int64(N_NODES) + s, kind="stable")
    grp_s = grp[order0]
    s_s = s[order0]
    gcounts = np.bincount(grp, minlength=N_CORES * N_TILES)
    gstart = np.zeros(N_CORES * N_TILES + 1, np.int64)
    np.cumsum(gcounts, out=gstart[1:])

    # cumulative mandatory counts: edges with src < WSTART[w+1] must be
    # assigned to windows <= w.  Template cumulative caps (shared across
    # cores) = max over cores, rounded up to whole chunks, floor 2 chunks per
    # window.
    cum_mand = np.zeros((N_CORES * N_TILES, N_WIN), np.int64)
    for g in range(N_CORES * N_TILES):
        a, e = gstart[g], gstart[g + 1]
        src = s_s[a:e]
        for w in range(N_WIN - 1):
            cum_mand[g, w] = np.searchsorted(src, WSTART[w + 1])
        cum_mand[g, N_WIN - 1] = e - a
    cm = cum_mand.reshape(N_CORES, N_TILES, N_WIN).max(axis=0)   # [98, 5]
    cumcap = -(-cm // P) * P
    for w in range(N_WIN):
        cumcap[:, w] = np.maximum(cumcap[:, w], 2 * P * (w + 1))
    for w in range(1, N_WIN):
        cumcap[:, w] = np.maximum(cumcap[:, w], cumcap[:, w - 1] + P)
    caps_ti = np.empty((N_TILES, N_WIN), np.int64)
    caps_ti[:, 0] = cumcap[:, 0]
    caps_ti[:, 1:] = cumcap[:, 1:] - cumcap[:, :-1]
    caps_ti[:, N_WIN - 1] = 1 << 30           # last window absorbs any spill

    bk_s = np.empty(E, np.int8)
    n_gw = np.zeros((N_CORES * N_TILES, N_WIN), np.int64)
    for g in range(N_CORES * N_TILES):
        a, e = gstart[g], gstart[g + 1]
        src = s_s[a:e]                       # sorted ascending within group
        caps = caps_ti[g % N_TILES]
        pos = 0
        n = e - a
        for w in range(N_WIN):
            hi = np.searchsorted(src, WSTART[w] + WLEN)
            take = min(int(caps[w]), hi - pos)
            if w + 1 < N_WIN:
                mand = np.searchsorted(src, WSTART[w + 1]) - pos
                assert mand <= caps[w], (g, w, mand, caps[w])
            else:
                take = n - pos
            bk_s[a + pos : a + pos + take] = w
            n_gw[g, w] = take
            pos += take
        assert pos == n

    # chunk template per (w, ti): measured per-core max, shared across cores
    nch_tb = (
        -(-n_gw.reshape(N_CORES, N_TILES, N_WIN).max(axis=0) // P)
    ).T.copy()                                # [5, 98]

    tb_gbase = np.zeros((N_WIN, N_TILES), np.int64)
    chunk_ti = []
    wbase = np.zeros(N_WIN, np.int64)
    wn = np.zeros(N_WIN, np.int64)
    gc = 0
    for w in range(N_WIN):
        gc = ((gc + CALL_CH - 1) // CALL_CH) * CALL_CH
        wbase[w] = gc
        for ti_ in range(N_TILES):
            tb_gbase[w, ti_] = gc
            gc += nch_tb[w, ti_]
            chunk_ti.extend([ti_] * int(nch_tb[w, ti_]))
        wn[w] = gc - wbase[w]
    ctot = ((gc + CALL_CH - 1) // CALL_CH) * CALL_CH

    # --- per-edge slot assignment ------------------------------------------
    core_s = core[order0]
    ti_s = ti[order0]
    dl_s = dl[order0]
    key = (core_s * N_WIN + bk_s) * N_TILES + ti_s
    order1 = np.argsort(key, kind="stable")
    key_s = key[order1]
    counts = np.bincount(key, minlength=N_CORES * N_WIN * N_TILES)
    starts = np.zeros(N_CORES * N_WIN * N_TILES + 1, np.int64)
    np.cumsum(counts, out=starts[1:])
    rank = np.arange(E, dtype=np.int64) - starts[key_s]
    core_f = core_s[order1]
    w_f = bk_s[order1].astype(np.int64)
    gpos = tb_gbase[w_f, ti_s[order1]] * P + rank
    sl = (s_s[order1] - np.asarray(WSTART, np.int64)[w_f]).astype(np.int16)

    idx_arr = np.zeros((N_CORES, 128, ctot * 8), np.int16)
    dl_arr = np.full((N_CORES, 128, ctot), 255.0, BF16)
    idx_arr[core_f, gpos % 16, gpos // 16] = sl
    dl_arr[core_f, gpos % 128, gpos // 128] = dl_s[order1].astype(BF16)

    ti_of_chunk = np.full(ctot, -1, np.int64)
    pos = 0
    for w in range(N_WIN):
        nb = int(wn[w])
        ti_of_chunk[int(wbase[w]) : int(wbase[w]) + nb] = chunk_ti[pos : pos + nb]
        pos += nb
    calls = []
    for w in range(N_WIN):
        nb = int(wn[w])
        for k in range((nb + CALL_CH - 1) // CALL_CH):
            c0 = int(wbase[w]) + CALL_CH * k
            nn = min(CALL_CH, nb - CALL_CH * k)
            calls.append((w, c0, nn, int(ti_of_chunk[c0])))
    # Q7 SWDGE reads the wrapped index block from each 16-partition group
    # (one per gpsimd core) -> replicate rows 0:16 into rows 16:128.
    idx_arr[:, 16:, :] = np.tile(idx_arr[:, :16, :], (1, 7, 1))

    wcols = {}
    for w in range(N_WIN):
        cs = [(c0, nn) for (ww, c0, nn, _) in calls if ww == w]
        lo = min(c for (c, _) in cs) * 8
        hi = max((c + n) for (c, n) in cs) * 8
        wcols[w] = (lo, hi)
    meta = dict(nch_tb=nch_tb, tb_gbase=tb_gbase, calls=calls, ctot=ctot, wcols=wcols)
    return idx_arr, dl_arr, meta


def _build(ctx, tc, aps, metas):
    import concourse.mybir as mybir

    nc = tc.nc
    f32 = mybir.dt.float32
    bf16 = mybir.dt.bfloat16
    i16 = mybir.dt.int16
    Alu = mybir.AluOpType
    Act = mybir.ActivationFunctionType

    cp = ctx.enter_context(tc.tile_pool(name="const", bufs=1))

    def load(name, dtype):
        ap = aps[name].ap()
        t = cp.tile(list(ap.shape), dtype, tag=name)
        nc.sync.dma_start(out=t[:], in_=ap[:])
        return t

    # Per-window idx tensors: dependency tracking is tile-granular, so one
    # big idx tensor would gate every gather on the full 2MB DMA.  Loading
    # window w's (small) tensor right before its calls lets the Q7 queues
    # ramp with the DMA stream.
    idxw_t = [[None] * N_WIN, [None] * N_WIN]
    for w in range(N_WIN):
        for d in (0, 1):
            idxw_t[d][w] = load(f"idxw{d}_{w}", i16)
    dl_t = [load("dl0", bf16), load("dl1", bf16)]
    iota_t = load("iota8", bf16)
    ident_t = load("ident", bf16)
    wh_t = [load("wh0", bf16), load("wh1", bf16)]
    bh_t = [load("bh0", bf16), load("bh1", bf16)]
    u_t = [load("u0", bf16), load("u1", bf16)]
    dvh_t = [load("dvh0", f32), load("dvh1", f32)]

    xb_ap = [aps["xb0"].ap(), aps["xb1"].ap()]
    xs_ap = aps["xs"].ap()
    out_ap = aps["out"].ap()

    gp = ctx.enter_context(tc.tile_pool(name="g", bufs=20))
    s8p = ctx.enter_context(tc.tile_pool(name="s8", bufs=16))
    xlp = ctx.enter_context(tc.tile_pool(name="xl", bufs=4))
    aggp = ctx.enter_context(tc.tile_pool(name="agg", bufs=4))
    rp = ctx.enter_context(tc.tile_pool(name="r", bufs=4))
    op_ = ctx.enter_context(tc.tile_pool(name="o", bufs=3))
    ps_t = ctx.enter_context(tc.tile_pool(name="psT", bufs=4, space="PSUM"))
    ps_b = ctx.enter_context(tc.tile_pool(name="psB", bufs=2, space="PSUM"))

    # --- emit all gather calls in consumption order -------------------------
    all_calls = []
    for d in (0, 1):
        for (w, c0, nn, fti) in metas[d]["calls"]:
            all_calls.append((fti, d, w, c0, nn))
    all_calls.sort()

    # one shared register per distinct num_idxs value: a fresh to_reg per call
    # would put 250 MOVEs on the serial Pool stream (~15us)
    nregs = {}

    def nreg(n):
        r = nregs.get(n)
        if r is None:
            r = nc.gpsimd.to_reg(n)
            nregs[n] = r
        return r

    G = [{}, {}]
    qctr = 0
    for (fti, d, w, c0, nn) in all_calls:
        g = gp.tile([128, nn * 128], bf16, tag="g", name="g")
        lo = metas[d]["wcols"][w][0]
        nc.gpsimd.dma_gather(
            out_ap=g[:].rearrange("p (c e) -> p c e", e=128),
            in_ap=xb_ap[d][WSTART[w] : WSTART[w] + WLEN, :],
            idxs_ap=idxw_t[d][w][:, c0 * 8 - lo : (c0 + nn) * 8 - lo],
            num_idxs=nn * 128,
            num_idxs_reg=nreg(nn * 128),
            elem_size=128,
            queue_num=qctr % 4,
        )
        qctr += 1
        G[d][c0 // CALL_CH] = g

    # --- main tile loop -----------------------------------------------------
    S8 = [{}, {}]

    def get_s8(d, batch):
        t = S8[d].get(batch)
        if t is None:
            t = s8p.tile([128, 1024], bf16, tag="s8", name="s8")
            nc.vector.tensor_tensor(
                out=t[:].rearrange("p (c e) -> p c e", e=128),
                in0=iota_t[:].rearrange("p (c e) -> p c e", e=128),
                in1=dl_t[d][:, batch * 8 : batch * 8 + 8]
                .unsqueeze(2)
                .broadcast_to([128, 8, 128]),
                op=Alu.is_equal,
            )
            S8[d][batch] = t
        return t

    for ti in range(N_TILES):
        r_ = [None, None]
        xsl = xlp.tile([128, 2 * D], bf16, tag="xl")
        nc.sync.dma_start(out=xsl[:], in_=xs_ap[ti * P : (ti + 1) * P, :])
        for d in (0, 1):
            m = metas[d]
            total_ch = int(m["nch_tb"][:, ti].sum())
            psT = ps_t.tile([D, 128], f32, tag="psT")
            nc.tensor.matmul(
                out=psT[:], lhsT=xsl[:, d * D : (d + 1) * D], rhs=ident_t[:],
                start=True, stop=(total_ch == 0),
            )
            done = 0
            for w in range(N_WIN):
                n = int(m["nch_tb"][w, ti])
                base = int(m["tb_gbase"][w, ti])
                for cc in range(n):
                    gc = base + cc
                    s8 = get_s8(d, gc // 8)
                    g = G[d][gc // CALL_CH]
                    col = (gc % CALL_CH) * 128
                    scol = (gc % 8) * 128
                    done += 1
                    nc.tensor.matmul(
                        out=psT[:],
                        lhsT=g[:, col : col + D],
                        rhs=s8[:, scol : scol + 128],
                        start=False, stop=(done == total_ch),
                    )
            aggT = aggp.tile([D, 128], bf16, tag="agg")
            nc.scalar.activation(out=aggT[:], in_=psT[:], func=Act.Copy)
            psB = ps_b.tile([128, D], f32, tag="psB")
            nc.tensor.matmul(
                out=psB[:], lhsT=aggT[:], rhs=wh_t[d][:], start=True, stop=False
            )
            nc.tensor.matmul(
                out=psB[:],
                lhsT=u_t[d][:, ti * P : (ti + 1) * P],
                rhs=bh_t[d][:],
                start=False, stop=True,
            )
            r_[d] = rp.tile([128, D], f32, name=f"r{d}", tag=f"r{d}")
            nc.scalar.activation(
                out=r_[d][:], in_=psB[:], func=Act.Relu,
                scale=dvh_t[d][:, ti : ti + 1],
            )
        o = op_.tile([128, D], f32, tag="o")
        nc.vector.tensor_add(out=o[:], in0=r_[0][:], in1=r_[1][:])
        nc.sync.dma_start(
            out=out_ap[ti * P : (ti + 1) * P, :], in_=o[:, :]
        )


def kernel(x, edge_index, W_f, b_f, W_b, b_b):
    global LAST_RESULTS
    import concourse.tile as tile
    from concourse import bacc, mybir
    from concourse import bass_utils

    x = np.asarray(x, dtype=np.float32)
    ei = np.asarray(edge_index).astype(np.int64)
    W_f = np.asarray(W_f, dtype=np.float32)
    b_f = np.asarray(b_f, dtype=np.float32)
    W_b = np.asarray(W_b, dtype=np.float32)
    b_b = np.asarray(b_b, dtype=np.float32)
    src, dst = ei[0], ei[1]

    ideg_f = np.bincount(dst, minlength=N_NODES)
    ideg_b = np.bincount(src, minlength=N_NODES)
    deg_f = (ideg_f + 1).astype(np.float32)
    deg_b = (ideg_b + 1).astype(np.float32)
    dinv_f = (1.0 / np.sqrt(deg_f)).astype(np.float32)
    dinv_b = (1.0 / np.sqrt(deg_b)).astype(np.float32)
    dinvs = [dinv_f, dinv_b]
    degs = [deg_f, deg_b]

    # balanced node -> slot permutation (shared by both directions)
    slot = _balance_nodes(np.stack([ideg_f, ideg_b]))

    # direction 0 (forward): messages src -> dst; direction 1: dst -> src
    prep = [_prep_dir(slot[dst], src), _prep_dir(slot[src], dst)]
    metas = [prep[0][2], prep[1][2]]

    # pre-scaled gather sources x~ = dinv * x (bf16, padded to 128 cols)
    # and permuted per-slot arrays
    occupied = np.zeros(N_CORES * TILE_PAD, bool)
    occupied[slot] = True
    node_of_slot = np.zeros(N_CORES * TILE_PAD, np.int64)
    node_of_slot[slot] = np.arange(N_NODES)

    xb = []
    u_arr = []
    dvh = []
    xself = np.zeros((N_CORES, TILE_PAD, 2 * D), dtype=BF16)
    for d in (0, 1):
        xt = (x * dinvs[d][:, None]).astype(BF16)
        xbd = np.zeros((N_NODES, 128), dtype=BF16)
        xbd[:, :D] = xt
        xb.append(xbd)
        slot_dinv = np.where(occupied, dinvs[d][node_of_slot], 0.0).astype(np.float32)
        slot_u = np.where(occupied, np.sqrt(degs[d][node_of_slot]), 0.0)
        xs_flat = np.zeros((N_CORES * TILE_PAD, D), dtype=BF16)
        xs_flat[occupied] = xt[node_of_slot[occupied]]
        xself[:, :, d * D : (d + 1) * D] = xs_flat.reshape(N_CORES, TILE_PAD, D)
        u_arr.append(slot_u.reshape(N_CORES, 1, TILE_PAD).astype(BF16))
        dvh.append(
            (0.5 * slot_dinv).reshape(N_CORES, N_TILES, 128).transpose(0, 2, 1).copy()
        )

    iota8 = np.tile(np.arange(128, dtype=np.float32), 8).reshape(1, 1024)
    iota8 = np.broadcast_to(iota8, (128, 1024)).astype(BF16).copy()
    ident = np.eye(128, dtype=np.float32).astype(BF16)
    whs = [W_f.astype(BF16), W_b.astype(BF16)]
    bhs = [b_f.reshape(1, D).astype(BF16), b_b.reshape(1, D).astype(BF16)]

    nc = bacc.Bacc(
        "TRN2",
        target_bir_lowering=False,
        debug=False,
        enable_asserts=False,
        num_devices=N_CORES,
        num_swdge_queues=4,
        dynamic_dma_scratch_size=49152,
    )
    dt = mybir.dt
    aps = {}
    aps["iota8"] = nc.dram_tensor("iota8", [128, 1024], dt.bfloat16, kind="ExternalInput")
    aps["ident"] = nc.dram_tensor("ident", [128, 128], dt.bfloat16, kind="ExternalInput")
    aps["xs"] = nc.dram_tensor("xs", [TILE_PAD, 2 * D], dt.bfloat16, kind="ExternalInput")
    for d in (0, 1):
        ct = metas[d]["ctot"]
        aps[f"xb{d}"] = nc.dram_tensor(f"xb{d}", [N_NODES, 128], dt.bfloat16, kind="ExternalInput")
        aps[f"wh{d}"] = nc.dram_tensor(f"wh{d}", [D, D], dt.bfloat16, kind="ExternalInput")
        aps[f"bh{d}"] = nc.dram_tensor(f"bh{d}", [1, D], dt.bfloat16, kind="ExternalInput")
        aps[f"u{d}"] = nc.dram_tensor(f"u{d}", [1, TILE_PAD], dt.bfloat16, kind="ExternalInput")
        aps[f"dvh{d}"] = nc.dram_tensor(f"dvh{d}", [128, N_TILES], dt.float32, kind="ExternalInput")
        for w in range(N_WIN):
            lo, hi = metas[d]["wcols"][w]
            aps[f"idxw{d}_{w}"] = nc.dram_tensor(
                f"idxw{d}_{w}", [128, hi - lo], dt.int16, kind="ExternalInput"
            )
        aps[f"dl{d}"] = nc.dram_tensor(f"dl{d}", [128, ct], dt.bfloat16, kind="ExternalInput")
    aps["out"] = nc.dram_tensor("out", [TILE_PAD, D], dt.float32, kind="ExternalOutput")

    with tile.TileContext(nc) as tc, ExitStack() as ctx:
        _build(ctx, tc, aps, metas)
    nc.compile()

    in_maps = []
    for c in range(N_CORES):
        m = {"iota8": iota8, "ident": ident, "xs": xself[c]}
        for d in (0, 1):
            idx_arr, dl_arr, _ = prep[d]
            m[f"xb{d}"] = xb[d]
            m[f"wh{d}"] = whs[d]
            m[f"bh{d}"] = bhs[d]
            m[f"u{d}"] = u_arr[d][c]
            m[f"dvh{d}"] = dvh[d][c]
            for w in range(N_WIN):
                lo, hi = metas[d]["wcols"][w]
                m[f"idxw{d}_{w}"] = np.ascontiguousarray(idx_arr[c][:, lo:hi])
            m[f"dl{d}"] = dl_arr[c]
        in_maps.append(m)

    LAST_RESULTS = bass_utils.run_bass_kernel_spmd(
        nc, in_maps, core_ids=list(range(N_CORES))
    )
    allout = np.concatenate([r["out"] for r in LAST_RESULTS.results], axis=0)
    return allout[slot].astype(np.float32)
